# revision 1
# baseline (speedup 1.0000x reference)
"""ONIMemoryHub kernel for 8 Trainium2 NeuronCores (Bass/Tile).

Sharding: data-parallel over batch for the query side; episodic store and
semantic memory sharded across cores for the key/value projections, with
AllGathers of the projected (normalized, pre-scaled) keys/values.

kernel(**inputs) takes FULL inputs (as produced by reference.setup_inputs())
and returns the FULL [4096, 2048] output.
"""
import math

import numpy as np

import concourse.bass as bass
import concourse.mybir as mybir
import concourse.tile as tile
from concourse import bacc
from concourse.bass_utils import run_bass_kernel_spmd
from concourse.masks import make_identity

AF = mybir.ActivationFunctionType
AXL = mybir.AxisListType
ALU = mybir.AluOpType

NCORES = 8
B, H, N, M, S = 4096, 2048, 4096, 16384, 64
BL, NL, ML = B // NCORES, N // NCORES, M // NCORES   # 512, 512, 2048
HT = H // 128                                        # 16 h-tiles
P = 128
NBT = BL // P                                        # 4 b-tiles
EP_K = 8
SEM_K = 4
LN_EPS = 1e-5
RECENCY = 0.01   # 1 - RECENCY_DECAY

F32 = mybir.dt.float32
F32R = mybir.dt.float32r
U32 = mybir.dt.uint32

# dtype knobs (iterate on these for perf; F32 = exact)
SIM_DT = F32     # sim matmul inputs (qT/qsT/keys)
PROJ_DT = F32    # q/qs/ek/ks projection inputs
VAL_DT = F32     # value-side + output projections


def build():
    nc = bacc.Bacc("TRN2", target_bir_lowering=False, debug=False,
                   num_devices=NCORES)

    def din(name, shape, dt=F32):
        return nc.dram_tensor(name, shape, dt, kind="ExternalInput").ap()

    # per-core slices
    query_s = din("query_s", [BL, H])
    ep_s = din("ep_s", [NL, H])
    semk_s = din("semk_s", [ML, H])
    ep_imp_s = din("ep_imp_s", [NL])
    ep_ts_s = din("ep_ts_s", [NL])
    # replicated
    ep_imp = din("ep_imp", [N])
    ep_ts = din("ep_ts", [N])
    sem_values = din("sem_values", [M, H])
    W_query = din("W_query", [H, H])
    W_ek = din("W_ek", [H, H])
    W_ev = din("W_ev", [H, H])
    W_eo = din("W_eo", [H, H])
    W_sq = din("W_sq", [H, H])
    W_sk = din("W_sk", [H, H])
    W_so = din("W_so", [H, H])
    W_ro = din("W_ro", [H, H])
    work_slots = din("work_slots", [S, H])
    gate_W1 = din("gate_W1", [H, 64])
    gate_b1 = din("gate_b1", [64])
    gate_W2 = din("gate_W2", [64, 3])
    gate_b2 = din("gate_b2", [3])
    ln_gamma = din("ln_gamma", [H])
    ln_beta = din("ln_beta", [H])

    out_s = nc.dram_tensor("out_s", [BL, H], F32, kind="ExternalOutput").ap()

    with tile.TileContext(nc) as tc:
        with (
            tc.tile_pool(name="cst", bufs=1) as cst,
            tc.tile_pool(name="big", bufs=1) as big,
            tc.tile_pool(name="rows", bufs=1) as rows,
            tc.tile_pool(name="ld", bufs=2) as ld,
            tc.tile_pool(name="s512", bufs=3) as s512p,
            tc.tile_pool(name="wcol", bufs=2) as wcolp,
            tc.tile_pool(name="wtile", bufs=3) as wtp,
            tc.tile_pool(name="sm", bufs=1) as sm,
            tc.tile_pool(name="tiny", bufs=2) as tiny,
            tc.tile_pool(name="simb", bufs=3) as simb,
            tc.tile_pool(name="gath", bufs=2) as gath,
            tc.tile_pool(name="ps_tr", bufs=2, space="PSUM") as ps_tr,
            tc.tile_pool(name="ps_mm", bufs=4, space="PSUM") as ps_mm,
            tc.tile_pool(name="ps_sml", bufs=2, space="PSUM") as ps_sml,
            tc.tile_pool(name="dram", bufs=1, space="DRAM") as dram,
        ):
            ident = cst.tile([P, P], F32)
            make_identity(nc, ident[:])
            ones_col = cst.tile([P, 1], F32)
            nc.vector.memset(ones_col[:], 1.0)

            # ---------- helpers ----------
            # big slot chains (explicit liveness via shared tags):
            #   xTin: epT -> skT(x4) -> qTin -> qsT
            #   kT  : ekT -> ksT(x4) -> qT
            def load_transposed(src_ap, rows_n, name, tag):
                """src [rows_n, H] DRAM -> xT tile [128, HT, rows_n]."""
                nt = rows_n // P
                xT = big.tile([P, HT, 512], PROJ_DT, tag=tag, name=name)
                for t in range(nt):
                    nat = ld.tile([P, H], F32, tag="nat", name="nat")
                    nc.sync.dma_start(nat[:], src_ap[t * P:(t + 1) * P, :])
                    for hi in range(HT):
                        pst = ps_tr.tile([P, P], F32, tag="tr", name="trp")
                        nc.tensor.transpose(out=pst[:],
                                            in_=nat[:, hi * P:(hi + 1) * P],
                                            identity=ident[:])
                        nc.vector.tensor_copy(xT[:, hi, t * P:(t + 1) * P], pst[:])
                return xT

            def wcol_tile(w_ap, j, dt):
                t = wcolp.tile([P, HT, P], dt, tag="wcol", name="wcol")
                nc.sync.dma_start(
                    t[:],
                    w_ap[:, j * P:(j + 1) * P].rearrange("(hi p) c -> p hi c", p=P))
                return t

            def project_T(xT, w_ap, name, tag, out_dt):
                """yT[h_out, 512] = (x @ W).T given xT [128, HT, 512]."""
                yT = big.tile([P, HT, 512], out_dt, tag=tag, name=name)
                for j in range(HT):
                    wc = wcol_tile(w_ap, j, xT.dtype)
                    pst = ps_mm.tile([P, 512], F32, tag="mm", name="projps")
                    for hi in range(HT):
                        nc.tensor.matmul(
                            pst[:], wc[:, hi, :], xT[:, hi, :],
                            start=(hi == 0), stop=(hi == HT - 1))
                    nc.vector.tensor_copy(yT[:, j, :], pst[:])
                return yT

            def inv_norm_row(xT, extra_row=None):
                """[1, 512] = (extra or 1)/max(||x_col||,1e-12); xT [128,HT,512]."""
                row = rows.tile([1, 512], F32, tag="nrow", name="nrow", bufs=2)
                sq = s512p.tile([P, 512], F32, tag="s512", name="sqn")
                psn = ps_sml.tile([1, 512], F32, tag="sml", name="npsum")
                for hi in range(HT):
                    nc.scalar.square(sq[:, :], xT[:, hi, :])
                    nc.tensor.matmul(
                        psn[:1, :], ones_col[:], sq[:, :],
                        start=(hi == 0), stop=(hi == HT - 1))
                nc.vector.tensor_copy(row[:1, :], psn[:1, :])
                nc.scalar.sqrt(row[:1, :], row[:1, :])
                nc.vector.tensor_scalar_max(row[:1, :], row[:1, :], 1e-12)
                nc.vector.reciprocal(row[:1, :], row[:1, :])
                if extra_row is not None:
                    nc.vector.tensor_mul(row[:1, :], row[:1, :], extra_row)
                return row

            def scale_cols(xT, scale_row):
                bc = s512p.tile([P, 512], F32, tag="s512", name="bcn")
                nc.gpsimd.partition_broadcast(bc[:, :], scale_row[:1, :])
                for hi in range(HT):
                    nc.vector.tensor_mul(xT[:, hi, :], xT[:, hi, :], bc[:, :])

            # ===================================================================
            # Phase W: episodic recency/importance weights
            # ===================================================================
            def rec_weight(imp_ap, ts_ap, shape, tagb):
                """(1+imp)*exp(-|1-ts|*RECENCY) elementwise; returns tile."""
                impt = rows.tile(shape, F32, tag=tagb + "i", name="impt")
                tst = rows.tile(shape, F32, tag=tagb + "t", name="tst")
                nc.sync.dma_start(impt[:shape[0], :], imp_ap)
                nc.sync.dma_start(tst[:shape[0], :], ts_ap)
                s = tst[:shape[0], :]
                nc.scalar.activation(s, s, AF.Copy, bias=0.0, scale=-1.0)
                nc.vector.tensor_scalar_add(s, s, 1.0)
                nc.scalar.activation(s, s, AF.Abs)
                nc.scalar.activation(s, s, AF.Exp, scale=-RECENCY)
                si = impt[:shape[0], :]
                nc.vector.tensor_scalar_add(si, si, 1.0)
                nc.vector.tensor_mul(si, si, s)
                return impt

            # global sum in [128, 32] layout
            wfull = rec_weight(ep_imp.rearrange("(p c) -> p c", p=P),
                               ep_ts.rearrange("(p c) -> p c", p=P),
                               [P, N // P], "wf")
            wpart = rows.tile([P, 1], F32, tag="wpart", name="wpart")
            nc.vector.reduce_sum(wpart[:, :], wfull[:, :], axis=AXL.X)
            pssum = ps_sml.tile([1, 512], F32, tag="sml", name="wsps")
            nc.tensor.matmul(pssum[:1, :1], ones_col[:], wpart[:, :],
                             start=True, stop=True)
            wsum = rows.tile([1, 1], F32, tag="wsum", name="wsum")
            nc.vector.tensor_copy(wsum[:1, :], pssum[:1, :1])
            nc.vector.tensor_scalar_add(wsum[:1, :], wsum[:1, :], 1e-8)
            nc.vector.reciprocal(wsum[:1, :], wsum[:1, :])
            # local slice weights [1, NL], normalized
            wloc = rec_weight(ep_imp_s[None, :], ep_ts_s[None, :], [1, NL], "wl")
            nc.vector.tensor_scalar(wloc[:1, :], wloc[:1, :], wsum[:1, :1], None,
                                    op0=ALU.mult)

            # ===================================================================
            # Phase M: sharded memory-side projections + AllGathers
            # ===================================================================
            ag_nek_in = dram.tile([H, NL], F32, name="ag_nek_in")
            ag_nek_out = dram.tile([NCORES * H, NL], F32, addr_space="Shared",
                                   name="ag_nek_out")
            ag_ev_in = dram.tile([NL, H], F32, name="ag_ev_in")
            ag_ev_out = dram.tile([N, H], F32, addr_space="Shared",
                                  name="ag_ev_out")
            ag_nks_in = [dram.tile([H, 512], F32, name=f"ag_nks_in{i}")
                         for i in range(ML // 512)]
            ag_nks_out = [dram.tile([NCORES * H, 512], F32,
                                    addr_space="Shared", name=f"ag_nks_out{i}")
                          for i in range(ML // 512)]

            # --- episodic: transpose slice, project keys/vals ---
            epT = load_transposed(ep_s, NL, "epT", "xTin")
            ekT = project_T(epT, W_ek, "ekT", "kT", SIM_DT)
            inv_ek = inv_norm_row(ekT, extra_row=wloc[:1, :])
            scale_cols(ekT, inv_ek)
            for hi in range(HT):
                nc.sync.dma_start(ag_nek_in[hi * P:(hi + 1) * P, :], ekT[:, hi, :])
            # e_vals natural layout [NL, H]
            for nt in range(NL // P):
                for jc in range(H // 512):
                    pst = ps_mm.tile([P, 512], F32, tag="mm", name="evps")
                    for hi in range(HT):
                        wt = wtp.tile([P, 512], F32, tag="wt", name="wtev")
                        nc.sync.dma_start(
                            wt[:],
                            W_ev[hi * P:(hi + 1) * P, jc * 512:(jc + 1) * 512])
                        nc.tensor.matmul(
                            pst[:], epT[:, hi, nt * P:(nt + 1) * P], wt[:],
                            start=(hi == 0), stop=(hi == HT - 1))
                    evs = s512p.tile([P, 512], F32, tag="s512", name="evout")
                    nc.vector.tensor_copy(evs[:], pst[:])
                    nc.sync.dma_start(
                        ag_ev_in[nt * P:(nt + 1) * P, jc * 512:(jc + 1) * 512],
                        evs[:])

            # --- semantic keys: 4 chunks of 512 ---
            for mc in range(ML // 512):
                skT = load_transposed(
                    semk_s[mc * 512:(mc + 1) * 512, :], 512, f"skT{mc}", "xTin")
                ksT = project_T(skT, W_sk, f"ksT{mc}", "kT", SIM_DT)
                inv_ks = inv_norm_row(ksT)
                scale_cols(ksT, inv_ks)
                for hi in range(HT):
                    nc.sync.dma_start(
                        ag_nks_in[mc][hi * P:(hi + 1) * P, :],
                        ksT[:, hi, :])
                nc.gpsimd.collective_compute(
                    "AllGather", ALU.bypass,
                    replica_groups=[list(range(NCORES))],
                    ins=[ag_nks_in[mc].opt()], outs=[ag_nks_out[mc].opt()])

            for ag_i, ag_o in ((ag_nek_in, ag_nek_out), (ag_ev_in, ag_ev_out)):
                nc.gpsimd.collective_compute(
                    "AllGather", ALU.bypass,
                    replica_groups=[list(range(NCORES))],
                    ins=[ag_i.opt()], outs=[ag_o.opt()])

            # --- pre-convert output-projection weights to f32r in DRAM ---
            weo_r = dram.tile([H, H], F32R, name="weo_r")
            wso_r = dram.tile([H, H], F32R, name="wso_r")
            wro_r = dram.tile([H, H], F32R, name="wro_r")
            for w_ap, dst in ((W_eo, weo_r), (W_so, wso_r), (W_ro, wro_r)):
                for ti in range(HT):
                    for tj in range(H // 512):
                        wcv = wtp.tile([P, 512], F32, tag="wt", name="wcv")
                        nc.sync.dma_start(
                            wcv[:],
                            w_ap[ti * P:(ti + 1) * P, tj * 512:(tj + 1) * 512])
                        wcr = s512p.tile([P, 512], F32R, tag="s512", name="wcr")
                        nc.vector.tensor_copy(wcr[:], wcv[:])
                        nc.sync.dma_start(
                            dst[ti * P:(ti + 1) * P, tj * 512:(tj + 1) * 512],
                            wcr[:])



            # ===================================================================
            # Phase Q: query-side projections + work/gate precompute
            # ===================================================================
            qTin = load_transposed(query_s, BL, "qTin", "xTin")
            qT = project_T(qTin, W_query, "qT", "kT", F32)
            qsT = project_T(qT, W_sq, "qsT", "xTin", SIM_DT)
            inv_q = inv_norm_row(qT)
            inv_qs = inv_norm_row(qsT)

            # transpose inv rows -> per-partition [128, NBT] via DRAM bounce
            invq_p = cst.tile([P, NBT], F32, name="invq_p")
            invqs_p = cst.tile([P, NBT], F32, name="invqs_p")
            bounce = dram.tile([2, BL], F32, name="bounce")
            nc.sync.dma_start(bounce[0:1, :], inv_q[:1, :])
            nc.sync.dma_start(bounce[1:2, :], inv_qs[:1, :])
            nc.sync.dma_start(
                invq_p[:, :], bounce[0:1, :].rearrange("o (t p) -> (o p) t", p=P))
            nc.sync.dma_start(
                invqs_p[:, :], bounce[1:2, :].rearrange("o (t p) -> (o p) t", p=P))

            # --- work slots transposed + gate weights ---
            wsT = big.tile([P, HT, S], F32, name="wsT")
            for hi in range(HT):
                wsn = s512p.tile([S, 512], F32, tag="s512", name="wsn")
                nc.sync.dma_start(wsn[:S, :P], work_slots[:, hi * P:(hi + 1) * P])
                pst = ps_tr.tile([P, S], F32, tag="tr", name="wstp")
                nc.tensor.transpose(out=pst[:, :S], in_=wsn[:S, :P],
                                    identity=ident[:S, :S])
                nc.vector.tensor_copy(wsT[:, hi, :], pst[:, :S])
            gw1 = big.tile([P, HT, 64], F32, name="gw1")
            nc.sync.dma_start(gw1[:], gate_W1.rearrange("(hi p) c -> p hi c", p=P))
            gw2 = cst.tile([64, 3], F32, name="gw2")
            nc.sync.dma_start(gw2[:, :], gate_W2)

            def bcast_row(dram_row, width, pool, tag, name):
                row = rows.tile([1, width], F32, tag="crow", name="crow", bufs=2)
                nc.sync.dma_start(row[:1, :], dram_row)
                t = pool.tile([P, width], F32, tag=tag, name=name)
                nc.gpsimd.partition_broadcast(t[:, :], row[:1, :])
                return t

            b1bc = bcast_row(gate_b1[None, :], 64, cst, "", "b1bc")
            b2bc = bcast_row(gate_b2[None, :], 3, cst, "", "b2bc")

            inv_sqrt_h = 1.0 / math.sqrt(H)
            ewT_pre = []
            gw_pre = []
            for bt in range(NBT):
                # work attention probs (transposed) precompute
                psw = ps_sml.tile([P, S], F32, tag="sml", name="pswk")
                for hi in range(HT):
                    nc.tensor.matmul(
                        psw[:, :S], qT[:, hi, bt * P:(bt + 1) * P], wsT[:, hi, :],
                        start=(hi == 0), stop=(hi == HT - 1))
                wmax = tiny.tile([P, 1], F32, tag="c1", name="wmax")
                nc.vector.reduce_max(wmax[:, :], psw[:, :S], axis=AXL.X)
                nc.vector.tensor_scalar_mul(wmax[:, :], wmax[:, :], -inv_sqrt_h)
                ew = tiny.tile([P, S], F32, tag="c64", name="ew")
                nc.scalar.activation(ew[:, :], psw[:, :S], AF.Exp,
                                     bias=wmax[:, :1], scale=inv_sqrt_h)
                zw = tiny.tile([P, 1], F32, tag="c1", name="zw")
                nc.vector.reduce_sum(zw[:, :], ew[:, :], axis=AXL.X)
                nc.vector.reciprocal(zw[:, :], zw[:, :])
                nc.vector.tensor_scalar(ew[:, :], ew[:, :], zw[:, :1], None,
                                        op0=ALU.mult)
                pset = ps_tr.tile([S, P], F32, tag="tr", name="ewtp")
                nc.tensor.transpose(out=pset[:S, :], in_=ew[:, :],
                                    identity=ident[:])
                ewT = cst.tile([S, P], F32, name=f"ewT{bt}")
                nc.vector.tensor_copy(ewT[:, :], pset[:S, :])
                ewT_pre.append(ewT)

                # gate
                psg = ps_sml.tile([P, 64], F32, tag="sml", name="psg")
                for hi in range(HT):
                    nc.tensor.matmul(
                        psg[:, :64], qT[:, hi, bt * P:(bt + 1) * P], gw1[:, hi, :],
                        start=(hi == 0), stop=(hi == HT - 1))
                hid = tiny.tile([P, 64], F32, tag="c64", name="hid")
                nc.vector.tensor_add(hid[:, :], psg[:, :64], b1bc[:, :])
                nc.scalar.activation(hid[:, :], hid[:, :], AF.Silu)
                psht = ps_tr.tile([64, P], F32, tag="tr", name="hidtp")
                nc.tensor.transpose(out=psht[:64, :], in_=hid[:, :],
                                    identity=ident[:])
                hidT = tiny.tile([64, P], F32, tag="c128", name="hidT")
                nc.vector.tensor_copy(hidT[:, :], psht[:64, :])
                psg2 = ps_sml.tile([P, 3], F32, tag="sml", name="psg2")
                nc.tensor.matmul(psg2[:, :3], hidT[:, :], gw2[:, :],
                                 start=True, stop=True)
                gl = cst.tile([P, 3], F32, name=f"gl{bt}")
                nc.vector.tensor_add(gl[:, :], psg2[:, :3], b2bc[:, :])
                gmax = tiny.tile([P, 1], F32, tag="c1", name="gmax")
                nc.vector.reduce_max(gmax[:, :], gl[:, :], axis=AXL.X)
                nc.vector.tensor_scalar_mul(gmax[:, :], gmax[:, :], -1.0)
                nc.scalar.activation(gl[:, :], gl[:, :], AF.Exp, bias=gmax[:, :1])
                gz = tiny.tile([P, 1], F32, tag="c1", name="gz")
                nc.vector.reduce_sum(gz[:, :], gl[:, :], axis=AXL.X)
                nc.vector.reciprocal(gz[:, :], gz[:, :])
                nc.vector.tensor_scalar(gl[:, :], gl[:, :], gz[:, :1], None,
                                        op0=ALU.mult)
                gw_pre.append(gl)

            # ===================================================================
            # Phase S: similarity + per-chunk top-8 candidates
            # ===================================================================
            cand_v_e = [big.tile([P, (N // 512) * 8], F32, tag=f"cve{bt}",
                                 name=f"cve{bt}") for bt in range(NBT)]
            cand_i_e = [big.tile([P, (N // 512) * 8], F32, tag=f"cie{bt}",
                                 name=f"cie{bt}") for bt in range(NBT)]
            cand_v_s = [big.tile([P, (M // 512) * 8], F32, tag=f"cvs{bt}",
                                 name=f"cvs{bt}") for bt in range(NBT)]
            cand_i_s = [big.tile([P, (M // 512) * 8], F32, tag=f"cis{bt}",
                                 name=f"cis{bt}") for bt in range(NBT)]

            def sim_phase(xT, key_dram, nchunks, cand_v, cand_i, rank_rows):
                for ch in range(nchunks):
                    r = (ch * 512) // rank_rows
                    sub = (ch * 512) % rank_rows
                    if isinstance(key_dram, list):
                        kd, sub = key_dram[sub // 512], 0
                    else:
                        kd = key_dram
                    psts = [ps_mm.tile([P, 512], F32, tag="mm", name=f"simps{i}")
                            for i in range(NBT)]
                    for hi in range(HT):
                        kt = s512p.tile([P, 512], SIM_DT, tag="s512", name="keyt")
                        nc.sync.dma_start(
                            kt[:],
                            kd[r * H + hi * P: r * H + (hi + 1) * P,
                               sub:sub + 512])
                        for bt in range(NBT):
                            nc.tensor.matmul(
                                psts[bt][:],
                                xT[:, hi, bt * P:(bt + 1) * P],
                                kt[:],
                                start=(hi == 0), stop=(hi == HT - 1))
                    for bt in range(NBT):
                        sc = simb.tile([P, 512], F32, tag="simc", name="simc")
                        nc.vector.tensor_copy(sc[:], psts[bt][:])
                        mx = simb.tile([P, 8], F32, tag="mx", name="mx")
                        mi = simb.tile([P, 8], U32, tag="mi", name="mi")
                        nc.vector.max(out=mx[:], in_=sc[:])
                        nc.vector.max_index(out=mi[:], in_max=mx[:], in_values=sc[:])
                        nc.vector.tensor_copy(cand_v[bt][:, ch * 8:(ch + 1) * 8],
                                              mx[:])
                        mif = simb.tile([P, 8], F32, tag="mif", name="mif")
                        nc.vector.tensor_copy(mif[:], mi[:])
                        nc.vector.tensor_scalar_add(
                            cand_i[bt][:, ch * 8:(ch + 1) * 8], mif[:],
                            float(ch * 512))

            sim_phase(qT, ag_nek_out, N // 512, cand_v_e, cand_i_e, NL)

            sim_phase(qsT, ag_nks_out, M // 512, cand_v_s, cand_i_s, ML)

            # ===================================================================
            # Phase F: per-b-tile merge, softmax, gather-attend, blend, out
            # ===================================================================
            def topk_attend(cand_v, cand_i, k, inv_p, bt, vals_dram, gscale,
                            acc_tag):
                """Merged top-k -> softmax (x gscale) -> gather + weighted sum."""
                top8 = tiny.tile([P, 8], F32, tag="c8", name="top8")
                nc.vector.max(out=top8[:], in_=cand_v[:])
                idxf = tiny.tile([P, 8], F32, tag="c8", name="idxf")
                eqm = sm.tile([P, 256], F32, tag="eqm", name="eqm")
                for kk in range(k):
                    w = cand_v.shape[-1]
                    nc.vector.tensor_scalar(
                        eqm[:, :w], cand_v[:], top8[:, kk:kk + 1], None,
                        op0=ALU.is_equal)
                    nc.vector.tensor_tensor(
                        out=eqm[:, :w], in0=eqm[:, :w], in1=cand_i[:], op=ALU.mult)
                    nc.vector.reduce_sum(idxf[:, kk:kk + 1], eqm[:, :w], axis=AXL.X)
                idxu = tiny.tile([P, 8], U32, tag="c8u", name="idxu")
                nc.vector.tensor_copy(idxu[:, :k], idxf[:, :k])
                sc8 = tiny.tile([P, 8], F32, tag="c8", name="sc8")
                nc.vector.tensor_scalar(
                    sc8[:, :k], top8[:, :k], inv_p[:, bt:bt + 1], None,
                    op0=ALU.mult)
                negm = tiny.tile([P, 1], F32, tag="c1", name="negm")
                nc.vector.tensor_scalar_mul(negm[:, :], sc8[:, 0:1], -1.0)
                nc.scalar.activation(sc8[:, :k], sc8[:, :k], AF.Exp,
                                     bias=negm[:, :1])
                zs = tiny.tile([P, 1], F32, tag="c1", name="zs")
                nc.vector.reduce_sum(zs[:, :], sc8[:, :k], axis=AXL.X)
                nc.vector.reciprocal(zs[:, :], zs[:, :])
                nc.vector.tensor_scalar(zs[:, :], zs[:, :], gscale, None,
                                        op0=ALU.mult)
                nc.vector.tensor_scalar(sc8[:, :k], sc8[:, :k], zs[:, :1], None,
                                        op0=ALU.mult)
                acc = sm.tile([P, H], F32, tag=acc_tag, name="acc" + acc_tag)
                nc.vector.memset(acc[:, :], 0.0)
                for kk in range(k):
                    g = gath.tile([P, H], F32, tag="g", name="g")
                    nc.gpsimd.indirect_dma_start(
                        out=g[:, :], out_offset=None, in_=vals_dram,
                        in_offset=bass.IndirectOffsetOnAxis(
                            ap=idxu[:, kk:kk + 1], axis=0))
                    nc.vector.scalar_tensor_tensor(
                        out=acc[:, :], in0=g[:, :], scalar=sc8[:, kk:kk + 1],
                        in1=acc[:, :], op0=ALU.mult, op1=ALU.add)
                return acc

            def transpose_128xH(src, dt=F32):
                dst = sm.tile([P, HT, P], dt, tag="scr8k", name="tr8k")
                for hi in range(HT):
                    pst = ps_tr.tile([P, P], F32, tag="tr", name="trf")
                    nc.tensor.transpose(out=pst[:], in_=src[:, hi * P:(hi + 1) * P],
                                        identity=ident[:])
                    nc.vector.tensor_copy(dst[:, hi, :], pst[:])
                return dst

            for bt in range(NBT):
                gl = gw_pre[bt]
                # attends: fold gate weight into softmax normalization
                acc_e = topk_attend(cand_v_e[bt][:], cand_i_e[bt][:], EP_K,
                                    invq_p, bt, ag_ev_out[:, :], gl[:, 1:2],
                                    "sl1")
                acc_s = topk_attend(cand_v_s[bt][:], cand_i_s[bt][:], SEM_K,
                                    invqs_p, bt, sem_values, gl[:, 2:3], "sl2")

                # bl = gw0 * w_out
                bl = sm.tile([P, H], F32, tag="sl3", name="bl")
                for jc in range(H // 512):
                    wsn = s512p.tile([S, 512], F32, tag="s512", name="wsn2")
                    nc.sync.dma_start(wsn[:S, :],
                                      work_slots[:, jc * 512:(jc + 1) * 512])
                    psw2 = ps_mm.tile([P, 512], F32, tag="mm", name="psw2")
                    nc.tensor.matmul(psw2[:], ewT_pre[bt][:, :], wsn[:S, :],
                                     start=True, stop=True)
                    nc.vector.tensor_scalar(
                        bl[:, jc * 512:(jc + 1) * 512], psw2[:], gl[:, 0:1],
                        None, op0=ALU.mult)

                # bl += (acc @ W) with gate weight already in acc
                for acc, w_ap in ((acc_e, weo_r), (acc_s, wso_r)):
                    accT = transpose_128xH(acc, F32R)
                    for jc in range(H // 512):
                        pso = ps_mm.tile([P, 512], F32, tag="mm", name="pso")
                        for hi in range(HT):
                            wt = wtp.tile([P, 512], F32R, tag="wt", name="wtoo")
                            nc.sync.dma_start(
                                wt[:],
                                w_ap[hi * P:(hi + 1) * P, jc * 512:(jc + 1) * 512])
                            nc.tensor.matmul(pso[:], accT[:, hi, :], wt[:],
                                             start=(hi == 0), stop=(hi == HT - 1))
                        nc.vector.tensor_add(
                            bl[:, jc * 512:(jc + 1) * 512],
                            bl[:, jc * 512:(jc + 1) * 512], pso[:])

                # out = LN(bl @ W_ro) * gamma + beta
                blT = transpose_128xH(bl, F32R)
                xo = sm.tile([P, H], F32, tag="sl1", name="xo")
                for jc in range(H // 512):
                    pso = ps_mm.tile([P, 512], F32, tag="mm", name="psro")
                    for hi in range(HT):
                        wt = wtp.tile([P, 512], F32R, tag="wt", name="wtro")
                        nc.sync.dma_start(
                            wt[:],
                            wro_r[hi * P:(hi + 1) * P, jc * 512:(jc + 1) * 512])
                        nc.tensor.matmul(pso[:], blT[:, hi, :], wt[:],
                                         start=(hi == 0), stop=(hi == HT - 1))
                    nc.vector.tensor_copy(xo[:, jc * 512:(jc + 1) * 512], pso[:])
                mu = tiny.tile([P, 1], F32, tag="c1", name="mu")
                nc.vector.reduce_sum(mu[:, :], xo[:, :], axis=AXL.X)
                nc.vector.tensor_scalar_mul(mu[:, :], mu[:, :], -1.0 / H)
                nc.vector.tensor_scalar(xo[:, :], xo[:, :], mu[:, :1], None,
                                        op0=ALU.add)
                sqx = sm.tile([P, H], F32, tag="sl2", name="sqx")
                vs = tiny.tile([P, 1], F32, tag="c1", name="vs")
                nc.scalar.activation(sqx[:, :], xo[:, :], AF.Square,
                                     accum_out=vs[:, :1])
                nc.vector.tensor_scalar_mul(vs[:, :], vs[:, :], 1.0 / H)
                nc.vector.tensor_scalar_add(vs[:, :], vs[:, :], LN_EPS)
                nc.scalar.sqrt(vs[:, :], vs[:, :])
                nc.vector.reciprocal(vs[:, :], vs[:, :])
                nc.vector.tensor_scalar(xo[:, :], xo[:, :], vs[:, :1], None,
                                        op0=ALU.mult)
                for jc in range(H // 512):
                    gbch = s512p.tile([P, 512], F32, tag="s512", name="gbch")
                    grow = rows.tile([1, 512], F32, tag="crow", name="grow",
                                     bufs=2)
                    nc.sync.dma_start(grow[:1, :],
                                      ln_gamma[None, jc * 512:(jc + 1) * 512])
                    nc.gpsimd.partition_broadcast(gbch[:, :], grow[:1, :])
                    nc.vector.tensor_mul(xo[:, jc * 512:(jc + 1) * 512],
                                         xo[:, jc * 512:(jc + 1) * 512],
                                         gbch[:, :])
                    bbch = s512p.tile([P, 512], F32, tag="s512", name="bbch")
                    brow = rows.tile([1, 512], F32, tag="crow", name="brow",
                                     bufs=2)
                    nc.sync.dma_start(brow[:1, :],
                                      ln_beta[None, jc * 512:(jc + 1) * 512])
                    nc.gpsimd.partition_broadcast(bbch[:, :], brow[:1, :])
                    nc.vector.tensor_add(xo[:, jc * 512:(jc + 1) * 512],
                                         xo[:, jc * 512:(jc + 1) * 512],
                                         bbch[:, :])
                nc.sync.dma_start(out_s[bt * P:(bt + 1) * P, :], xo[:, :])

    nc.finalize()
    return nc


_NC_CACHE = None
LAST_EXEC_NS = None


def kernel(**inputs) -> np.ndarray:
    global _NC_CACHE
    if _NC_CACHE is None:
        _NC_CACHE = build()
    nc = _NC_CACHE

    def arr(x):
        return np.ascontiguousarray(np.asarray(x), dtype=np.float32)

    in_maps = []
    for c in range(NCORES):
        in_maps.append({
            "query_s": arr(inputs["query"][c * BL:(c + 1) * BL]),
            "ep_s": arr(inputs["ep_store"][c * NL:(c + 1) * NL]),
            "semk_s": arr(inputs["sem_keys"][c * ML:(c + 1) * ML]),
            "ep_imp_s": arr(inputs["ep_importance"][c * NL:(c + 1) * NL]),
            "ep_ts_s": arr(inputs["ep_timestamps"][c * NL:(c + 1) * NL]),
            "ep_imp": arr(inputs["ep_importance"]),
            "ep_ts": arr(inputs["ep_timestamps"]),
            "sem_values": arr(inputs["sem_values"]),
            "W_query": arr(inputs["W_query"]),
            "W_ek": arr(inputs["W_ek"]),
            "W_ev": arr(inputs["W_ev"]),
            "W_eo": arr(inputs["W_eo"]),
            "W_sq": arr(inputs["W_sq"]),
            "W_sk": arr(inputs["W_sk"]),
            "W_so": arr(inputs["W_so"]),
            "W_ro": arr(inputs["W_ro"]),
            "work_slots": arr(inputs["work_slots"]),
            "gate_W1": arr(inputs["gate_W1"]),
            "gate_b1": arr(inputs["gate_b1"]),
            "gate_W2": arr(inputs["gate_W2"]),
            "gate_b2": arr(inputs["gate_b2"]),
            "ln_gamma": arr(inputs["ln_gamma"]),
            "ln_beta": arr(inputs["ln_beta"]),
        })
    res = run_bass_kernel_spmd(nc, in_maps, core_ids=list(range(NCORES)))
    return np.concatenate([res.results[c]["out_s"] for c in range(NCORES)],
                          axis=0)



# revision 10
# speedup vs baseline: 1.4774x; 1.4774x over previous
"""ONIMemoryHub kernel for 8 Trainium2 NeuronCores (Bass/Tile).

Strategy (v2):
- Selection path (projections feeding top-k similarity + the similarity
  matmuls) runs as 3-term bf16 hi/lo splits: x@W = xh@Wh + xl@Wh + xh@Wl,
  ~2^-19 relative accuracy at 3 PE cycles/row (vs 4 for fp32).
- Values path (W_ev/W_eo/W_so/W_ro, work/gate) runs in plain bf16.
- Episodic: keys projected/normalized/weighted on the owning core, packed
  hi/lo and AllGathered; each core scans all N keys for its own queries.
  Top-k attend gathers RAW ep_store rows (replicated input) and applies
  W_ev @ W_eo after the weighted sum (linearity) - no value AllGather.
- Semantic: keys stay sharded; query projections (qs) are AllGathered
  (hi/lo packed); each core scans ALL queries against its local keys and
  takes local top-4 per query; an AllToAll returns every core's candidates
  for the queries each core owns; exact merge + softmax + gather of raw
  sem_values happens on the query owner. Per-key 1/||ks|| is applied to sim
  rows pre-top-k; per-query 1/||qs|| post-merge (order-invariant).
- Host precomputes transposes and bf16 hi/lo splits of inputs/weights.

kernel(**inputs) takes FULL inputs and returns the FULL [4096, 2048] output.
"""
import math

import numpy as np
import ml_dtypes

import concourse.bass as bass
import concourse.mybir as mybir
import concourse.tile as tile
from concourse import bacc
from concourse.bass_utils import run_bass_kernel_spmd
from concourse.masks import make_identity

AF = mybir.ActivationFunctionType
AXL = mybir.AxisListType
ALU = mybir.AluOpType

NCORES = 8
B, H, N, M, S = 4096, 2048, 4096, 16384, 64
BL, NL, ML = B // NCORES, N // NCORES, M // NCORES   # 512, 512, 2048
P = 128
HT = H // P                                          # 16
NBT = BL // P                                        # 4
EP_K = 8
SEM_K = 4
LN_EPS = 1e-5
RECENCY = 0.01

F32 = mybir.dt.float32
BF16 = mybir.dt.bfloat16
U32 = mybir.dt.uint32


def build():
    nc = bacc.Bacc("TRN2", target_bir_lowering=False, debug=False,
                   num_devices=NCORES)

    def din(name, shape, dt=F32):
        return nc.dram_tensor(name, shape, dt, kind="ExternalInput").ap()

    qtin = din("qtin", [P, 2, HT, BL], BF16)
    eptin = din("eptin", [P, 2, HT, NL], BF16)
    sktin = din("sktin", [P, 2, HT, ML], BF16)
    wq_t = din("wq_t", [HT, P, 2, HT, P], BF16)
    wek_t = din("wek_t", [HT, P, 2, HT, P], BF16)
    wsq_t = din("wsq_t", [HT, P, 2, HT, P], BF16)
    wsk_t = din("wsk_t", [HT, P, 2, HT, P], BF16)
    wev_t = din("wev_t", [4, P, HT, 512], BF16)
    weo_t = din("weo_t", [4, P, HT, 512], BF16)
    wso_t = din("wso_t", [4, P, HT, 512], BF16)
    wro_t = din("wro_t", [4, P, HT, 512], BF16)
    ep_store_b = din("ep_store_b", [N, H], BF16)
    sem_values_b = din("sem_values_b", [M, H], BF16)
    wsT_b = din("wsT_b", [P, HT, S], BF16)
    work_b = din("work_b", [S, H], BF16)
    gw1_b = din("gw1_b", [P, HT, 64], BF16)
    gw2_b = din("gw2_b", [64, 3], BF16)
    ep_imp = din("ep_imp", [N])
    ep_ts = din("ep_ts", [N])
    ep_imp_s = din("ep_imp_s", [NL])
    ep_ts_s = din("ep_ts_s", [NL])
    gate_b1 = din("gate_b1", [64])
    gate_b2 = din("gate_b2", [3])
    ln_gamma = din("ln_gamma", [H])
    ln_beta = din("ln_beta", [H])
    key_base = din("key_base", [1])

    out_s = nc.dram_tensor("out_s", [BL, H], F32, kind="ExternalOutput").ap()

    with tile.TileContext(nc) as tc:
        with (
            tc.tile_pool(name="cst", bufs=1) as cst,
            tc.tile_pool(name="rows", bufs=2) as rows,
            tc.tile_pool(name="wcol", bufs=2) as wcolp,
            tc.tile_pool(name="sq", bufs=2) as sqp,
            tc.tile_pool(name="simc", bufs=2) as simcp,
            tc.tile_pool(name="tiny", bufs=2) as tiny,
            tc.tile_pool(name="gath", bufs=2) as gath,
            tc.tile_pool(name="ps_mm", bufs=3, space="PSUM") as ps_mm,
            tc.tile_pool(name="ps_tr", bufs=1, space="PSUM") as ps_tr,
            tc.tile_pool(name="ps_sml", bufs=2, space="PSUM") as ps_sml,
            tc.tile_pool(name="dram", bufs=1, space="DRAM") as dram,
        ):
            ident = cst.tile([P, P], F32)
            make_identity(nc, ident[:])
            ident_b = cst.tile([P, P], BF16)
            nc.vector.tensor_copy(ident_b[:], ident[:])
            ones_col = cst.tile([P, 1], F32)
            nc.vector.memset(ones_col[:], 1.0)

            ag_ek_in = dram.tile([2 * H, NL], BF16, name="ag_ek_in")
            ag_ek_out = dram.tile([NCORES * 2 * H, NL], BF16,
                                  addr_space="Shared", name="ag_ek_out")
            ag_qs_in = dram.tile([2 * H, BL], BF16, name="ag_qs_in")
            ag_qs_out = dram.tile([NCORES * 2 * H, BL], BF16,
                                  addr_space="Shared", name="ag_qs_out")
            ks_dram = dram.tile([2 * H, ML], BF16, name="ks_dram")
            cand_in = dram.tile([B, 8], F32, name="cand_in")
            cand_out = dram.tile([B, 8], F32, name="cand_out")
            bounce = dram.tile([2, BL], F32, name="bounce")

            # ---------- helpers ----------
            def load_wcol(w_ap, j):
                t = wcolp.tile([P, 2, HT, P], BF16, tag="wcol", name="wcol")
                nc.sync.dma_start(t[:], w_ap[j])
                return t

            def mm3(ps, stat, mov, s_sl=slice(None), m_sl=slice(None)):
                """ps = sum_hi [ Sh.T Mh + Sl.T Mh + Sh.T Ml ]."""
                for hi in range(HT):
                    sh = stat[:, 0, hi, s_sl]
                    sl = stat[:, 1, hi, s_sl]
                    mh = mov[:, 0, hi, m_sl]
                    ml = mov[:, 1, hi, m_sl]
                    nc.tensor.matmul(ps, sh, mh, start=(hi == 0), stop=False)
                    nc.tensor.matmul(ps, sl, mh, start=False, stop=False)
                    nc.tensor.matmul(ps, sh, ml, start=False,
                                     stop=(hi == HT - 1))

            def finish_inv_row(psn, width, extra_row=None):
                row = rows.tile([1, 512], F32, tag="nrow", name="nrow")
                nc.vector.tensor_copy(row[:1, :width], psn[:1, :width])
                nc.scalar.sqrt(row[:1, :width], row[:1, :width])
                nc.vector.tensor_scalar_max(row[:1, :width], row[:1, :width],
                                            1e-12)
                nc.vector.reciprocal(row[:1, :width], row[:1, :width])
                if extra_row is not None:
                    nc.vector.tensor_mul(row[:1, :width], row[:1, :width],
                                         extra_row)
                return row

            def bcast_row_dram(dram_row, width, name):
                row = rows.tile([1, width], F32, tag="crow", name="crow")
                nc.sync.dma_start(row[:1, :], dram_row)
                t = cst.tile([P, width], F32, name=name)
                nc.gpsimd.partition_broadcast(t[:, :], row[:1, :])
                return t

            # =================================================================
            # Phase W: episodic recency/importance weights
            # =================================================================
            def rec_weight(imp_ap, ts_ap, shape, tagb):
                impt = rows.tile(shape, F32, tag=tagb + "i", name="impt")
                tst = rows.tile(shape, F32, tag=tagb + "t", name="tst")
                nc.sync.dma_start(impt[:shape[0], :], imp_ap)
                nc.sync.dma_start(tst[:shape[0], :], ts_ap)
                s = tst[:shape[0], :]
                nc.scalar.activation(s, s, AF.Copy, bias=0.0, scale=-1.0)
                nc.vector.tensor_scalar_add(s, s, 1.0)
                nc.scalar.activation(s, s, AF.Abs)
                nc.scalar.activation(s, s, AF.Exp, scale=-RECENCY)
                si = impt[:shape[0], :]
                nc.vector.tensor_scalar_add(si, si, 1.0)
                nc.vector.tensor_mul(si, si, s)
                return impt

            wfull = rec_weight(ep_imp.rearrange("(p c) -> p c", p=P),
                               ep_ts.rearrange("(p c) -> p c", p=P),
                               [P, N // P], "wf")
            wpart = rows.tile([P, 1], F32, tag="wpart", name="wpart")
            nc.vector.reduce_sum(wpart[:, :], wfull[:, :], axis=AXL.X)
            pssum = ps_sml.tile([1, 512], F32, tag="nrm", name="wsps", bufs=1)
            nc.tensor.matmul(pssum[:1, :1], ones_col[:], wpart[:, :],
                             start=True, stop=True)
            wsum = rows.tile([1, 1], F32, tag="wsum", name="wsum")
            nc.vector.tensor_copy(wsum[:1, :], pssum[:1, :1])
            nc.vector.tensor_scalar_add(wsum[:1, :], wsum[:1, :], 1e-8)
            nc.vector.reciprocal(wsum[:1, :], wsum[:1, :])
            wloc = rec_weight(ep_imp_s[None, :], ep_ts_s[None, :], [1, NL],
                              "wl")
            nc.vector.tensor_scalar(wloc[:1, :], wloc[:1, :], wsum[:1, :1],
                                    None, op0=ALU.mult)

            # =================================================================
            # Phase EK: project episodic keys, scale by w/||k||, split, AG
            # =================================================================
            with tc.tile_pool(name="ph_ek", bufs=1) as ph_ek:
                ept = ph_ek.tile([P, 2, HT, NL], BF16, tag="ept", name="ept")
                nc.sync.dma_start(ept[:], eptin)
                ekf = ph_ek.tile([P, HT, NL], F32, tag="ekf", name="ekf")
                psn_ek = ps_sml.tile([1, 512], F32, tag="nrm", name="psn_ek",
                                     bufs=1)
                for j in range(HT):
                    wc = load_wcol(wek_t, j)
                    ps = ps_mm.tile([P, 512], F32, tag="mm", name="ps_ek")
                    mm3(ps[:], wc, ept)
                    nc.vector.tensor_copy(ekf[:, j, :], ps[:])
                    sq = sqp.tile([P, 512], F32, tag="sq", name="sq_ek")
                    nc.scalar.square(sq[:, :], ps[:])
                    nc.tensor.matmul(psn_ek[:1, :], ones_col[:], sq[:, :],
                                     start=(j == 0), stop=(j == HT - 1))
                inv_ek = finish_inv_row(psn_ek, NL, extra_row=wloc[:1, :])
                bc_ek = sqp.tile([P, 512], F32, tag="sq", name="bc_ek")
                nc.gpsimd.partition_broadcast(bc_ek[:, :], inv_ek[:1, :])
                ek_hl = ph_ek.tile([P, 2, HT, NL], BF16, tag="ekhl",
                                   name="ek_hl")
                for j in range(HT):
                    t = sqp.tile([P, 512], F32, tag="sq", name="t_ek")
                    nc.vector.tensor_mul(t[:, :], ekf[:, j, :], bc_ek[:, :])
                    nc.scalar.activation(ek_hl[:, 0, j, :], t[:, :], AF.Copy)
                    nc.vector.tensor_sub(ek_hl[:, 1, j, :], t[:, :],
                                         ek_hl[:, 0, j, :])
                nc.sync.dma_start(
                    ag_ek_in[0:H, :].rearrange("(hi p) c -> p hi c", p=P),
                    ek_hl[:, 0, :, :])
                nc.sync.dma_start(
                    ag_ek_in[H:2 * H, :].rearrange("(hi p) c -> p hi c", p=P),
                    ek_hl[:, 1, :, :])
                nc.gpsimd.collective_compute(
                    "AllGather", ALU.bypass,
                    replica_groups=[list(range(NCORES))],
                    ins=[ag_ek_in.opt()], outs=[ag_ek_out.opt()])

            with tc.tile_pool(name="ph_acc", bufs=1) as ph_acc:
                with tc.tile_pool(name="ph_qhl", bufs=1) as ph_qhl:
                    # =========================================================
                    # Phase Q: project queries, split (unscaled), norms
                    # =========================================================
                    q_hl = ph_qhl.tile([P, 2, HT, BL], BF16, tag="qhl",
                                       name="q_hl")
                    with tc.tile_pool(name="ph_qt", bufs=1) as ph_qt:
                        qt = ph_qt.tile([P, 2, HT, BL], BF16, tag="qt",
                                        name="qt")
                        nc.sync.dma_start(qt[:], qtin)
                        psn_q = ps_sml.tile([1, 512], F32, tag="nrm",
                                            name="psn_q", bufs=1)
                        for j in range(HT):
                            wc = load_wcol(wq_t, j)
                            ps = ps_mm.tile([P, 512], F32, tag="mm",
                                            name="ps_q")
                            mm3(ps[:], wc, qt)
                            nc.scalar.activation(q_hl[:, 0, j, :], ps[:],
                                                 AF.Copy)
                            nc.vector.tensor_sub(q_hl[:, 1, j, :], ps[:],
                                                 q_hl[:, 0, j, :])
                            sq = sqp.tile([P, 512], F32, tag="sq", name="sq_q")
                            nc.scalar.square(sq[:, :], ps[:])
                            nc.tensor.matmul(psn_q[:1, :], ones_col[:],
                                             sq[:, :], start=(j == 0),
                                             stop=(j == HT - 1))
                        inv_q = finish_inv_row(psn_q, BL)
                        nc.sync.dma_start(bounce[0:1, :], inv_q[:1, :])

                    # =========================================================
                    # Phase QS: semantic query projection (unscaled)
                    # =========================================================
                    with tc.tile_pool(name="ph_qs", bufs=1) as ph_qs:
                        qs_hl = ph_qs.tile([P, 2, HT, BL], BF16, tag="qshl",
                                           name="qs_hl")
                        psn_qs = ps_sml.tile([1, 512], F32, tag="nrm",
                                             name="psn_qs", bufs=1)
                        for j in range(HT):
                            wc = load_wcol(wsq_t, j)
                            ps = ps_mm.tile([P, 512], F32, tag="mm",
                                            name="ps_qs")
                            mm3(ps[:], wc, q_hl)
                            nc.scalar.activation(qs_hl[:, 0, j, :], ps[:],
                                                 AF.Copy)
                            nc.vector.tensor_sub(qs_hl[:, 1, j, :], ps[:],
                                                 qs_hl[:, 0, j, :])
                            sq = sqp.tile([P, 512], F32, tag="sq",
                                          name="sq_qs")
                            nc.scalar.square(sq[:, :], ps[:])
                            nc.tensor.matmul(psn_qs[:1, :], ones_col[:],
                                             sq[:, :], start=(j == 0),
                                             stop=(j == HT - 1))
                        inv_qs = finish_inv_row(psn_qs, BL)
                        nc.sync.dma_start(bounce[1:2, :], inv_qs[:1, :])
                        nc.sync.dma_start(
                            ag_qs_in[0:H, :].rearrange("(hi p) c -> p hi c",
                                                       p=P),
                            qs_hl[:, 0, :, :])
                        nc.sync.dma_start(
                            ag_qs_in[H:2 * H, :].rearrange(
                                "(hi p) c -> p hi c", p=P),
                            qs_hl[:, 1, :, :])
                        nc.gpsimd.collective_compute(
                            "AllGather", ALU.bypass,
                            replica_groups=[list(range(NCORES))],
                            ins=[ag_qs_in.opt()], outs=[ag_qs_out.opt()])

                    invq_p = cst.tile([P, NBT], F32, name="invq_p")
                    invqs_p = cst.tile([P, NBT], F32, name="invqs_p")
                    nc.sync.dma_start(
                        invq_p[:, :],
                        bounce[0:1, :].rearrange("o (t p) -> (o p) t", p=P))
                    nc.sync.dma_start(
                        invqs_p[:, :],
                        bounce[1:2, :].rearrange("o (t p) -> (o p) t", p=P))

                    # --- work attention + gate precompute ---
                    wsT = cst.tile([P, HT, S], BF16, name="wsT")
                    nc.sync.dma_start(wsT[:], wsT_b)
                    gw1 = cst.tile([P, HT, 64], BF16, name="gw1")
                    nc.sync.dma_start(gw1[:], gw1_b)
                    gw2 = cst.tile([64, 3], BF16, name="gw2")
                    nc.sync.dma_start(gw2[:, :], gw2_b)
                    b1bc = bcast_row_dram(gate_b1[None, :], 64, "b1bc")
                    b2bc = bcast_row_dram(gate_b2[None, :], 3, "b2bc")
                    kb_bc = bcast_row_dram(key_base[None, :], 1, "kb_bc")

                    inv_sqrt_h = 1.0 / math.sqrt(H)
                    ewT_pre = []
                    gw_pre = []
                    for bt in range(NBT):
                        qsl = slice(bt * P, (bt + 1) * P)
                        psw = ps_sml.tile([P, S], F32, tag="sml", name="pswk")
                        for hi in range(HT):
                            nc.tensor.matmul(
                                psw[:, :S], q_hl[:, 0, hi, qsl], wsT[:, hi, :],
                                start=(hi == 0), stop=(hi == HT - 1))
                        wmax = tiny.tile([P, 1], F32, tag="c1", name="wmax")
                        nc.vector.reduce_max(wmax[:, :], psw[:, :S],
                                             axis=AXL.X)
                        nc.vector.tensor_scalar_mul(wmax[:, :], wmax[:, :],
                                                    -inv_sqrt_h)
                        ew = tiny.tile([P, S], F32, tag="c64", name="ew")
                        nc.scalar.activation(ew[:, :], psw[:, :S], AF.Exp,
                                             bias=wmax[:, :1],
                                             scale=inv_sqrt_h)
                        zw = tiny.tile([P, 1], F32, tag="c1", name="zw")
                        nc.vector.reduce_sum(zw[:, :], ew[:, :], axis=AXL.X)
                        nc.vector.reciprocal(zw[:, :], zw[:, :])
                        nc.vector.tensor_scalar(ew[:, :], ew[:, :],
                                                zw[:, :1], None, op0=ALU.mult)
                        pset = ps_tr.tile([S, P], F32, tag="tr", name="ewtp")
                        nc.tensor.transpose(out=pset[:S, :], in_=ew[:, :],
                                            identity=ident[:])
                        ewT = cst.tile([S, P], BF16, name=f"ewT{bt}")
                        nc.vector.tensor_copy(ewT[:, :], pset[:S, :])
                        ewT_pre.append(ewT)

                        psg = ps_sml.tile([P, 64], F32, tag="sml", name="psg")
                        for hi in range(HT):
                            nc.tensor.matmul(
                                psg[:, :64], q_hl[:, 0, hi, qsl],
                                gw1[:, hi, :],
                                start=(hi == 0), stop=(hi == HT - 1))
                        hid = tiny.tile([P, 64], F32, tag="c64", name="hid")
                        nc.vector.tensor_add(hid[:, :], psg[:, :64],
                                             b1bc[:, :])
                        nc.scalar.activation(hid[:, :], hid[:, :], AF.Silu)
                        psht = ps_tr.tile([64, P], F32, tag="tr", name="hidtp")
                        nc.tensor.transpose(out=psht[:64, :], in_=hid[:, :],
                                            identity=ident[:])
                        hidT = tiny.tile([64, P], BF16, tag="c128",
                                         name="hidT")
                        nc.vector.tensor_copy(hidT[:, :], psht[:64, :])
                        psg2 = ps_sml.tile([P, 3], F32, tag="sml", name="psg2")
                        nc.tensor.matmul(psg2[:, :3], hidT[:, :], gw2[:, :],
                                         start=True, stop=True)
                        gl = cst.tile([P, 3], F32, name=f"gl{bt}")
                        nc.vector.tensor_add(gl[:, :], psg2[:, :3], b2bc[:, :])
                        gmax = tiny.tile([P, 1], F32, tag="c1", name="gmax")
                        nc.vector.reduce_max(gmax[:, :], gl[:, :], axis=AXL.X)
                        nc.vector.tensor_scalar_mul(gmax[:, :], gmax[:, :],
                                                    -1.0)
                        nc.scalar.activation(gl[:, :], gl[:, :], AF.Exp,
                                             bias=gmax[:, :1])
                        gz = tiny.tile([P, 1], F32, tag="c1", name="gz")
                        nc.vector.reduce_sum(gz[:, :], gl[:, :], axis=AXL.X)
                        nc.vector.reciprocal(gz[:, :], gz[:, :])
                        nc.vector.tensor_scalar(gl[:, :], gl[:, :],
                                                gz[:, :1], None, op0=ALU.mult)
                        gw_pre.append(gl)

                    # =========================================================
                    # Phase KS: project semantic keys, split -> DRAM; norms
                    # =========================================================
                    bc_ks = [cst.tile([P, 512], F32, name=f"bc_ks{kc}")
                             for kc in range(4)]
                    with tc.tile_pool(name="ph_ks", bufs=1) as ph_ks:
                        for mc in range(ML // 512):
                            msl = slice(mc * 512, (mc + 1) * 512)
                            skt = ph_ks.tile([P, 2, HT, 512], BF16, tag="skt",
                                             name="skt")
                            nc.sync.dma_start(skt[:],
                                              sktin[:, :, :, msl])
                            psn = ps_sml.tile([1, 512], F32, tag="nrm",
                                              name="psn_ks", bufs=1)
                            for j in range(HT):
                                wc = load_wcol(wsk_t, j)
                                ps = ps_mm.tile([P, 512], F32, tag="mm",
                                                name="ps_ks")
                                mm3(ps[:], wc, skt)
                                st = sqp.tile([P, 2, 512], BF16, tag="ksst",
                                              name="ksst")
                                nc.scalar.activation(st[:, 0, :], ps[:],
                                                     AF.Copy)
                                nc.vector.tensor_sub(st[:, 1, :], ps[:],
                                                     st[:, 0, :])
                                nc.sync.dma_start(
                                    ks_dram[j * P:(j + 1) * P, msl],
                                    st[:, 0, :])
                                nc.sync.dma_start(
                                    ks_dram[H + j * P:H + (j + 1) * P, msl],
                                    st[:, 1, :])
                                sq = sqp.tile([P, 512], F32, tag="sq",
                                              name="sq_ks")
                                nc.scalar.square(sq[:, :], ps[:])
                                nc.tensor.matmul(psn[:1, :], ones_col[:],
                                                 sq[:, :], start=(j == 0),
                                                 stop=(j == HT - 1))
                            inv = finish_inv_row(psn, 512)
                            nc.gpsimd.partition_broadcast(bc_ks[mc][:, :],
                                                          inv[:1, :512])

                    # =========================================================
                    # Phase SIM-E: own queries x all episodic keys
                    # =========================================================
                    cand_v_e = [cst.tile([P, 128], F32, name=f"cve{bt}")
                                for bt in range(NBT)]
                    cand_i_e = [cst.tile([P, 128], F32, name=f"cie{bt}")
                                for bt in range(NBT)]
                    with tc.tile_pool(name="ph_se", bufs=2) as ph_se:
                        for slab in range(NCORES):
                            base = slab * 2 * H
                            for khalf in range(2):
                                csl = slice(khalf * 256, (khalf + 1) * 256)
                                ekg = ph_se.tile([P, 2, HT, 256], BF16,
                                                 tag="ekg", name="ekg")
                                nc.sync.dma_start(
                                    ekg[:, 0, :, :],
                                    ag_ek_out[base:base + H, csl].rearrange(
                                        "(hi p) c -> p hi c", p=P))
                                nc.sync.dma_start(
                                    ekg[:, 1, :, :],
                                    ag_ek_out[base + H:base + 2 * H,
                                              csl].rearrange(
                                        "(hi p) c -> p hi c", p=P))
                                cid = 2 * slab + khalf
                                for bt in range(NBT):
                                    qsl = slice(bt * P, (bt + 1) * P)
                                    ps = ps_mm.tile([P, 512], F32, tag="mm",
                                                    name="ps_se")
                                    mm3(ps[:, :256], q_hl, ekg, s_sl=qsl)
                                    sc = simcp.tile([P, 256], F32, tag="sime",
                                                    name="sc_e")
                                    nc.vector.tensor_copy(sc[:], ps[:, :256])
                                    mx = tiny.tile([P, 8], F32, tag="mx",
                                                   name="mx_e")
                                    mi = tiny.tile([P, 8], U32, tag="mi",
                                                   name="mi_e")
                                    nc.vector.max(out=mx[:], in_=sc[:])
                                    nc.vector.max_index(out=mi[:],
                                                        in_max=mx[:],
                                                        in_values=sc[:])
                                    nc.vector.tensor_copy(
                                        cand_v_e[bt][:,
                                                     cid * 8:(cid + 1) * 8],
                                        mx[:])
                                    mif = tiny.tile([P, 8], F32, tag="mif",
                                                    name="mif_e")
                                    nc.vector.tensor_copy(mif[:], mi[:])
                                    nc.vector.tensor_scalar_add(
                                        cand_i_e[bt][:,
                                                     cid * 8:(cid + 1) * 8],
                                        mif[:], float(cid * 256))

                    # --- episodic top-8 merge + gather + weighted sum ---
                    acc_e_b = [ph_acc.tile([P, H], BF16, tag=f"acce{bt}",
                                           name=f"acce{bt}")
                               for bt in range(NBT)]
                    for bt in range(NBT):
                        top8 = tiny.tile([P, 8], F32, tag="c8", name="top8")
                        nc.vector.max(out=top8[:], in_=cand_v_e[bt][:])
                        idxf = tiny.tile([P, 8], F32, tag="c8b", name="idxf")
                        eqm = simcp.tile([P, 128], F32, tag="eqm", name="eqm")
                        for kk in range(EP_K):
                            nc.vector.tensor_scalar(
                                eqm[:, :], cand_v_e[bt][:],
                                top8[:, kk:kk + 1], None, op0=ALU.is_equal)
                            nc.vector.tensor_tensor(
                                out=eqm[:, :], in0=eqm[:, :],
                                in1=cand_i_e[bt][:], op=ALU.mult)
                            nc.vector.reduce_sum(idxf[:, kk:kk + 1],
                                                 eqm[:, :], axis=AXL.X)
                        idxu = tiny.tile([P, 8], U32, tag="c8u", name="idxu")
                        nc.vector.tensor_copy(idxu[:, :], idxf[:, :])
                        sc8 = tiny.tile([P, 8], F32, tag="c8c", name="sc8")
                        nc.vector.tensor_scalar(
                            sc8[:, :], top8[:, :], invq_p[:, bt:bt + 1], None,
                            op0=ALU.mult)
                        negm = tiny.tile([P, 1], F32, tag="c1", name="negm")
                        nc.vector.tensor_scalar_mul(negm[:, :], sc8[:, 0:1],
                                                    -1.0)
                        nc.scalar.activation(sc8[:, :], sc8[:, :], AF.Exp,
                                             bias=negm[:, :1])
                        zs = tiny.tile([P, 1], F32, tag="c1", name="zs")
                        nc.vector.reduce_sum(zs[:, :], sc8[:, :], axis=AXL.X)
                        nc.vector.reciprocal(zs[:, :], zs[:, :])
                        nc.vector.tensor_scalar(zs[:, :], zs[:, :],
                                                gw_pre[bt][:, 1:2], None,
                                                op0=ALU.mult)
                        nc.vector.tensor_scalar(sc8[:, :], sc8[:, :],
                                                zs[:, :1], None, op0=ALU.mult)
                        acc = simcp.tile([P, H], F32, tag="acc", name="acc_e",
                                         bufs=1)
                        nc.vector.memset(acc[:, :], 0.0)
                        for kk in range(EP_K):
                            g = gath.tile([P, H], BF16, tag="g", name="g_e")
                            nc.gpsimd.indirect_dma_start(
                                out=g[:, :], out_offset=None, in_=ep_store_b,
                                in_offset=bass.IndirectOffsetOnAxis(
                                    ap=idxu[:, kk:kk + 1], axis=0))
                            nc.vector.scalar_tensor_tensor(
                                out=acc[:, :], in0=g[:, :],
                                scalar=sc8[:, kk:kk + 1],
                                in1=acc[:, :], op0=ALU.mult, op1=ALU.add)
                        nc.vector.tensor_copy(acc_e_b[bt][:, :], acc[:, :])

                # ==== ph_qhl closed: q_hl freed ====
                # =============================================================
                # Phase SIM-S: ALL queries x local semantic keys (kc-outer)
                # =============================================================
                cand_sv = cst.tile([P, 32 * 32], F32, name="cand_sv")
                cand_si = cst.tile([P, 32 * 32], F32, name="cand_si")
                with tc.tile_pool(name="ph_ss", bufs=1) as ph_ss:
                    for kc in range(4):
                        msl = slice(kc * 512, (kc + 1) * 512)
                        ksc = ph_ss.tile([P, 2, HT, 512], BF16, tag="ksc",
                                         name="ksc")
                        nc.sync.dma_start(
                            ksc[:, 0, :, :],
                            ks_dram[0:H, msl].rearrange("(hi p) c -> p hi c",
                                                        p=P))
                        nc.sync.dma_start(
                            ksc[:, 1, :, :],
                            ks_dram[H:2 * H, msl].rearrange(
                                "(hi p) c -> p hi c", p=P))
                        for rq2 in range(16):
                            slabq = rq2 // 2
                            base = slabq * 2 * H
                            col0 = (rq2 % 2) * 256
                            qsg = ph_ss.tile([P, 2, HT, 256], BF16, tag="qsg",
                                             name="qsg", bufs=2)
                            nc.sync.dma_start(
                                qsg[:, 0, :, :],
                                ag_qs_out[base:base + H,
                                          col0:col0 + 256].rearrange(
                                    "(hi p) c -> p hi c", p=P))
                            nc.sync.dma_start(
                                qsg[:, 1, :, :],
                                ag_qs_out[base + H:base + 2 * H,
                                          col0:col0 + 256].rearrange(
                                    "(hi p) c -> p hi c", p=P))
                            for rq in range(2):
                                rqt = rq2 * 2 + rq
                                qssl = slice(rq * P, (rq + 1) * P)
                                ps = ps_mm.tile([P, 512], F32, tag="mm",
                                                name="ps_ss")
                                mm3(ps[:], qsg, ksc, s_sl=qssl)
                                sc = simcp.tile([P, 512], F32, tag="scs",
                                                name="sc_s")
                                nc.vector.tensor_mul(sc[:, :], ps[:],
                                                     bc_ks[kc][:, :])
                                mx = tiny.tile([P, 8], F32, tag="mx",
                                               name="mx_s")
                                mi = tiny.tile([P, 8], U32, tag="mi",
                                               name="mi_s")
                                nc.vector.max(out=mx[:], in_=sc[:])
                                nc.vector.max_index(out=mi[:], in_max=mx[:],
                                                    in_values=sc[:])
                                wsl = slice(rqt * 32 + kc * 8,
                                            rqt * 32 + (kc + 1) * 8)
                                nc.vector.tensor_copy(cand_sv[:, wsl], mx[:])
                                mif = tiny.tile([P, 8], F32, tag="mif",
                                                name="mif_s")
                                nc.vector.tensor_copy(mif[:], mi[:])
                                nc.vector.tensor_scalar_add(
                                    cand_si[:, wsl], mif[:], float(kc * 512))

                # local top-4 per query, global index, ship via AllToAll
                for rqt in range(32):
                    wsl = slice(rqt * 32, (rqt + 1) * 32)
                    top8 = tiny.tile([P, 8], F32, tag="c8", name="top8l")
                    nc.vector.max(out=top8[:], in_=cand_sv[:, wsl])
                    idxf = tiny.tile([P, 8], F32, tag="c8b", name="idxfl")
                    eqm = simcp.tile([P, 32], F32, tag="eqs", name="eqml")
                    for kk in range(SEM_K):
                        nc.vector.tensor_scalar(
                            eqm[:, :], cand_sv[:, wsl], top8[:, kk:kk + 1],
                            None, op0=ALU.is_equal)
                        nc.vector.tensor_tensor(out=eqm[:, :], in0=eqm[:, :],
                                                in1=cand_si[:, wsl],
                                                op=ALU.mult)
                        nc.vector.reduce_sum(idxf[:, kk:kk + 1], eqm[:, :],
                                             axis=AXL.X)
                    p4 = tiny.tile([P, 8], F32, tag="p4", name="p4")
                    nc.vector.tensor_copy(p4[:, 0:4], top8[:, 0:4])
                    nc.vector.tensor_scalar(
                        p4[:, 4:8], idxf[:, 0:4], kb_bc[:, 0:1], None,
                        op0=ALU.add)
                    nc.sync.dma_start(cand_in[rqt * P:(rqt + 1) * P, :],
                                      p4[:, :])
                nc.gpsimd.collective_compute(
                    "AllToAll", ALU.bypass,
                    replica_groups=[list(range(NCORES))],
                    ins=[cand_in.opt()], outs=[cand_out.opt()])

                # =============================================================
                # Phase FINAL
                # =============================================================
                with tc.tile_pool(name="fin", bufs=1) as fin:
                    def transpose_b(src_b, dst):
                        for hi in range(HT):
                            pst = ps_tr.tile([P, P], BF16, tag="trb16",
                                             name="trp")
                            nc.tensor.transpose(
                                out=pst[:], in_=src_b[:, hi * P:(hi + 1) * P],
                                identity=ident_b[:])
                            nc.vector.tensor_copy(dst[:, hi, :], pst[:])

                    def val_stage(w_ap, accT_list, out_tiles, mode,
                                  gscale=None):
                        for jc in range(4):
                            wv = fin.tile([P, HT, 512], BF16, tag="wv",
                                          name="wv", bufs=1)
                            nc.sync.dma_start(wv[:], w_ap[jc])
                            jsl = slice(jc * 512, (jc + 1) * 512)
                            for bt in range(NBT):
                                ps = ps_mm.tile([P, 512], F32, tag="mm",
                                                name="ps_v")
                                for hi in range(HT):
                                    nc.tensor.matmul(
                                        ps[:], accT_list[bt][:, hi, :],
                                        wv[:, hi, :], start=(hi == 0),
                                        stop=(hi == HT - 1))
                                if mode == "set":
                                    nc.vector.tensor_copy(
                                        out_tiles[bt][:, jsl], ps[:])
                                else:
                                    nc.vector.tensor_add(
                                        out_tiles[bt][:, jsl],
                                        out_tiles[bt][:, jsl], ps[:])

                    # e chain: tmp_e = acc_e @ W_ev
                    accT = [fin.tile([P, HT, P], BF16, tag="accT",
                                     name=f"accT{bt}", bufs=4)
                            for bt in range(NBT)]
                    for bt in range(NBT):
                        transpose_b(acc_e_b[bt], accT[bt])
                    tmp_e = [fin.tile([P, H], BF16, tag="t16",
                                      name=f"tmpe{bt}", bufs=4)
                             for bt in range(NBT)]
                    val_stage(wev_t, accT, tmp_e, "set")
                    accT2 = [fin.tile([P, HT, P], BF16, tag="accT",
                                      name=f"accT2{bt}", bufs=4)
                             for bt in range(NBT)]
                    for bt in range(NBT):
                        transpose_b(tmp_e[bt], accT2[bt])

                    # bl = gl0 * w_out
                    bl = [fin.tile([P, H], F32, tag="f32b", name=f"bl{bt}",
                                   bufs=4)
                          for bt in range(NBT)]
                    for jc in range(4):
                        wvw = fin.tile([S, 512], BF16, tag="wvw", name="wvw",
                                       bufs=2)
                        nc.sync.dma_start(wvw[:S, :],
                                          work_b[:, jc * 512:(jc + 1) * 512])
                        jsl = slice(jc * 512, (jc + 1) * 512)
                        for bt in range(NBT):
                            ps = ps_mm.tile([P, 512], F32, tag="mm",
                                            name="ps_w")
                            nc.tensor.matmul(ps[:], ewT_pre[bt][:, :],
                                             wvw[:S, :], start=True,
                                             stop=True)
                            nc.vector.tensor_scalar(
                                bl[bt][:, jsl], ps[:], gw_pre[bt][:, 0:1],
                                None, op0=ALU.mult)

                    # bl += tmp_e @ W_eo
                    val_stage(weo_t, accT2, bl, "add")

                    # --- semantic merge + gather (after AllToAll) ---
                    acc_s_b = [ph_acc.tile([P, H], BF16, tag=f"accs{bt}",
                                           name=f"accs{bt}")
                               for bt in range(NBT)]
                    for bt in range(NBT):
                        c32v = simcp.tile([P, 32], F32, tag="eqs",
                                          name="c32v")
                        c32i = simcp.tile([P, 32], F32, tag="eqs2",
                                          name="c32i")
                        for r in range(NCORES):
                            c8 = tiny.tile([P, 8], F32, tag="p4", name="c8in")
                            nc.sync.dma_start(
                                c8[:, :],
                                cand_out[r * BL + bt * P:
                                         r * BL + (bt + 1) * P, :])
                            nc.vector.tensor_copy(c32v[:, r * 4:(r + 1) * 4],
                                                  c8[:, 0:4])
                            nc.vector.tensor_copy(c32i[:, r * 4:(r + 1) * 4],
                                                  c8[:, 4:8])
                        top8 = tiny.tile([P, 8], F32, tag="c8", name="top8s")
                        nc.vector.max(out=top8[:], in_=c32v[:])
                        idxf = tiny.tile([P, 8], F32, tag="c8b", name="idxfs")
                        eqs = simcp.tile([P, 32], F32, tag="eqs3", name="eqs")
                        for kk in range(SEM_K):
                            nc.vector.tensor_scalar(
                                eqs[:, :], c32v[:, :], top8[:, kk:kk + 1],
                                None, op0=ALU.is_equal)
                            nc.vector.tensor_tensor(out=eqs[:, :],
                                                    in0=eqs[:, :],
                                                    in1=c32i[:, :],
                                                    op=ALU.mult)
                            nc.vector.reduce_sum(idxf[:, kk:kk + 1],
                                                 eqs[:, :], axis=AXL.X)
                        idxu = tiny.tile([P, 8], U32, tag="c8u", name="idxus")
                        nc.vector.tensor_copy(idxu[:, 0:4], idxf[:, 0:4])
                        sc4 = tiny.tile([P, 4], F32, tag="c4", name="sc4")
                        nc.vector.tensor_scalar(
                            sc4[:, :], top8[:, 0:4], invqs_p[:, bt:bt + 1],
                            None, op0=ALU.mult)
                        negm = tiny.tile([P, 1], F32, tag="c1", name="negms")
                        nc.vector.tensor_scalar_mul(negm[:, :], sc4[:, 0:1],
                                                    -1.0)
                        nc.scalar.activation(sc4[:, :], sc4[:, :], AF.Exp,
                                             bias=negm[:, :1])
                        zs = tiny.tile([P, 1], F32, tag="c1", name="zss")
                        nc.vector.reduce_sum(zs[:, :], sc4[:, :], axis=AXL.X)
                        nc.vector.reciprocal(zs[:, :], zs[:, :])
                        nc.vector.tensor_scalar(zs[:, :], zs[:, :],
                                                gw_pre[bt][:, 2:3], None,
                                                op0=ALU.mult)
                        nc.vector.tensor_scalar(sc4[:, :], sc4[:, :],
                                                zs[:, :1], None, op0=ALU.mult)
                        acc = simcp.tile([P, H], F32, tag="acc", name="acc_s",
                                         bufs=1)
                        nc.vector.memset(acc[:, :], 0.0)
                        for kk in range(SEM_K):
                            g = gath.tile([P, H], BF16, tag="g", name="g_s")
                            nc.gpsimd.indirect_dma_start(
                                out=g[:, :], out_offset=None,
                                in_=sem_values_b,
                                in_offset=bass.IndirectOffsetOnAxis(
                                    ap=idxu[:, kk:kk + 1], axis=0))
                            nc.vector.scalar_tensor_tensor(
                                out=acc[:, :], in0=g[:, :],
                                scalar=sc4[:, kk:kk + 1],
                                in1=acc[:, :], op0=ALU.mult, op1=ALU.add)
                        nc.vector.tensor_copy(acc_s_b[bt][:, :], acc[:, :])

                    # bl += acc_s @ W_so
                    accT_s = [fin.tile([P, HT, P], BF16, tag="accT",
                                       name=f"accTs{bt}", bufs=4)
                              for bt in range(NBT)]
                    for bt in range(NBT):
                        transpose_b(acc_s_b[bt], accT_s[bt])
                    val_stage(wso_t, accT_s, bl, "add")

                    # xo = bl @ W_ro; out = LN(xo)*gamma+beta
                    blb = [fin.tile([P, H], BF16, tag="t16", name=f"blb{bt}",
                                    bufs=4)
                           for bt in range(NBT)]
                    for bt in range(NBT):
                        nc.vector.tensor_copy(blb[bt][:, :], bl[bt][:, :])
                    accT_bl = [fin.tile([P, HT, P], BF16, tag="accT",
                                        name=f"accTb{bt}", bufs=4)
                               for bt in range(NBT)]
                    for bt in range(NBT):
                        transpose_b(blb[bt], accT_bl[bt])
                    xo = [fin.tile([P, H], F32, tag="f32b", name=f"xo{bt}",
                                   bufs=4)
                          for bt in range(NBT)]
                    val_stage(wro_t, accT_bl, xo, "set")

                    for bt in range(NBT):
                        x = xo[bt]
                        mu = tiny.tile([P, 1], F32, tag="c1", name="mu")
                        nc.vector.reduce_sum(mu[:, :], x[:, :], axis=AXL.X)
                        nc.vector.tensor_scalar_mul(mu[:, :], mu[:, :],
                                                    -1.0 / H)
                        nc.vector.tensor_scalar(x[:, :], x[:, :], mu[:, :1],
                                                None, op0=ALU.add)
                        sqx = simcp.tile([P, H], F32, tag="acc", name="sqx",
                                         bufs=1)
                        vs = tiny.tile([P, 1], F32, tag="c1", name="vs")
                        nc.scalar.activation(sqx[:, :], x[:, :], AF.Square,
                                             accum_out=vs[:, :1])
                        nc.vector.tensor_scalar_mul(vs[:, :], vs[:, :],
                                                    1.0 / H)
                        nc.vector.tensor_scalar_add(vs[:, :], vs[:, :],
                                                    LN_EPS)
                        nc.scalar.sqrt(vs[:, :], vs[:, :])
                        nc.vector.reciprocal(vs[:, :], vs[:, :])
                        nc.vector.tensor_scalar(x[:, :], x[:, :], vs[:, :1],
                                                None, op0=ALU.mult)
                        for jc in range(4):
                            jsl = slice(jc * 512, (jc + 1) * 512)
                            gbch = sqp.tile([P, 512], F32, tag="sq",
                                            name="gbch")
                            grow = rows.tile([1, 512], F32, tag="crow",
                                             name="grow")
                            nc.sync.dma_start(grow[:1, :],
                                              ln_gamma[None, jsl])
                            nc.gpsimd.partition_broadcast(gbch[:, :],
                                                          grow[:1, :])
                            nc.vector.tensor_mul(x[:, jsl], x[:, jsl],
                                                 gbch[:, :])
                            bbch = sqp.tile([P, 512], F32, tag="sq",
                                            name="bbch")
                            brow = rows.tile([1, 512], F32, tag="crow",
                                             name="brow")
                            nc.sync.dma_start(brow[:1, :],
                                              ln_beta[None, jsl])
                            nc.gpsimd.partition_broadcast(bbch[:, :],
                                                          brow[:1, :])
                            nc.vector.tensor_add(x[:, jsl], x[:, jsl],
                                                 bbch[:, :])
                        nc.sync.dma_start(out_s[bt * P:(bt + 1) * P, :],
                                          x[:, :])

    nc.finalize()
    return nc


_NC_CACHE = None


def _bf16_split(x):
    h = x.astype(ml_dtypes.bfloat16)
    l = (x - h.astype(np.float32)).astype(ml_dtypes.bfloat16)
    return h, l


def _tile_sel_weight(w):
    """[H, H] f32 -> [j, p, 2, hi, 128] bf16 hi/lo tiled."""
    h, l = _bf16_split(w)
    out = np.empty((HT, P, 2, HT, P), dtype=ml_dtypes.bfloat16)
    hr = h.reshape(HT, P, HT, P)   # [hi, p, j, c]
    lr = l.reshape(HT, P, HT, P)
    out[:, :, 0] = hr.transpose(2, 1, 0, 3)
    out[:, :, 1] = lr.transpose(2, 1, 0, 3)
    return np.ascontiguousarray(out)


def _tile_val_weight(w):
    """[H, H] f32 -> [jc, p, hi, 512] bf16."""
    b = w.astype(ml_dtypes.bfloat16)
    r = b.reshape(HT, P, 4, 512)   # [hi, p, jc, c]
    return np.ascontiguousarray(r.transpose(2, 1, 0, 3))


def _split_T(x):
    """[R, H] f32 -> [p, 2, hi, R] bf16 (transposed hi/lo)."""
    h, l = _bf16_split(x)
    R = x.shape[0]
    out = np.empty((P, 2, HT, R), dtype=ml_dtypes.bfloat16)
    out[:, 0] = h.T.reshape(HT, P, R).transpose(1, 0, 2)
    out[:, 1] = l.T.reshape(HT, P, R).transpose(1, 0, 2)
    return np.ascontiguousarray(out)


def kernel(**inputs) -> np.ndarray:
    global _NC_CACHE
    if _NC_CACHE is None:
        _NC_CACHE = build()
    nc = _NC_CACHE

    f32 = lambda x: np.ascontiguousarray(np.asarray(x), dtype=np.float32)
    query = f32(inputs["query"])
    ep_store = f32(inputs["ep_store"])
    sem_keys = f32(inputs["sem_keys"])
    work_slots = f32(inputs["work_slots"])

    shared = {
        "wq_t": _tile_sel_weight(f32(inputs["W_query"])),
        "wek_t": _tile_sel_weight(f32(inputs["W_ek"])),
        "wsq_t": _tile_sel_weight(f32(inputs["W_sq"])),
        "wsk_t": _tile_sel_weight(f32(inputs["W_sk"])),
        "wev_t": _tile_val_weight(f32(inputs["W_ev"])),
        "weo_t": _tile_val_weight(f32(inputs["W_eo"])),
        "wso_t": _tile_val_weight(f32(inputs["W_so"])),
        "wro_t": _tile_val_weight(f32(inputs["W_ro"])),
        "ep_store_b": ep_store.astype(ml_dtypes.bfloat16),
        "sem_values_b": f32(inputs["sem_values"]).astype(ml_dtypes.bfloat16),
        "wsT_b": np.ascontiguousarray(
            work_slots.T.astype(ml_dtypes.bfloat16).reshape(HT, P, S)
            .transpose(1, 0, 2)),
        "work_b": work_slots.astype(ml_dtypes.bfloat16),
        "gw1_b": np.ascontiguousarray(
            f32(inputs["gate_W1"]).astype(ml_dtypes.bfloat16)
            .reshape(HT, P, 64).transpose(1, 0, 2)),
        "gw2_b": f32(inputs["gate_W2"]).astype(ml_dtypes.bfloat16),
        "ep_imp": f32(inputs["ep_importance"]),
        "ep_ts": f32(inputs["ep_timestamps"]),
        "gate_b1": f32(inputs["gate_b1"]),
        "gate_b2": f32(inputs["gate_b2"]),
        "ln_gamma": f32(inputs["ln_gamma"]),
        "ln_beta": f32(inputs["ln_beta"]),
    }

    in_maps = []
    for c in range(NCORES):
        m = dict(shared)
        m["qtin"] = _split_T(query[c * BL:(c + 1) * BL])
        m["eptin"] = _split_T(ep_store[c * NL:(c + 1) * NL])
        m["sktin"] = _split_T(sem_keys[c * ML:(c + 1) * ML])
        m["ep_imp_s"] = f32(inputs["ep_importance"][c * NL:(c + 1) * NL])
        m["ep_ts_s"] = f32(inputs["ep_timestamps"][c * NL:(c + 1) * NL])
        m["key_base"] = np.array([c * ML], dtype=np.float32)
        in_maps.append(m)

    res = run_bass_kernel_spmd(nc, in_maps, core_ids=list(range(NCORES)))
    return np.concatenate([res.results[c]["out_s"] for c in range(NCORES)],
                          axis=0)


# revision 12
# speedup vs baseline: 1.5663x; 1.0602x over previous
"""ONIMemoryHub kernel for 8 Trainium2 NeuronCores (Bass/Tile).

Strategy (v2):
- Selection path (projections feeding top-k similarity + the similarity
  matmuls) runs as 3-term bf16 hi/lo splits: x@W = xh@Wh + xl@Wh + xh@Wl,
  ~2^-19 relative accuracy at 3 PE cycles/row (vs 4 for fp32).
- Values path (W_ev/W_eo/W_so/W_ro, work/gate) runs in plain bf16.
- Episodic: keys projected/normalized/weighted on the owning core, packed
  hi/lo and AllGathered; each core scans all N keys for its own queries.
  Top-k attend gathers RAW ep_store rows (replicated input) and applies
  W_ev @ W_eo after the weighted sum (linearity) - no value AllGather.
- Semantic: keys stay sharded; query projections (qs) are AllGathered
  (hi/lo packed); each core scans ALL queries against its local keys and
  takes local top-4 per query; an AllToAll returns every core's candidates
  for the queries each core owns; exact merge + softmax + gather of raw
  sem_values happens on the query owner. Per-key 1/||ks|| is applied to sim
  rows pre-top-k; per-query 1/||qs|| post-merge (order-invariant).
- Host precomputes transposes and bf16 hi/lo splits of inputs/weights.

kernel(**inputs) takes FULL inputs and returns the FULL [4096, 2048] output.
"""
import math

import numpy as np
import ml_dtypes

import concourse.bass as bass
import concourse.mybir as mybir
import concourse.tile as tile
from concourse import bacc
from concourse.bass_utils import run_bass_kernel_spmd
from concourse.masks import make_identity

AF = mybir.ActivationFunctionType
AXL = mybir.AxisListType
ALU = mybir.AluOpType

NCORES = 8
B, H, N, M, S = 4096, 2048, 4096, 16384, 64
BL, NL, ML = B // NCORES, N // NCORES, M // NCORES   # 512, 512, 2048
P = 128
HT = H // P                                          # 16
NBT = BL // P                                        # 4
EP_K = 8
SEM_K = 4
LN_EPS = 1e-5
RECENCY = 0.01

F32 = mybir.dt.float32
BF16 = mybir.dt.bfloat16
U32 = mybir.dt.uint32


def build():
    nc = bacc.Bacc("TRN2", target_bir_lowering=False, debug=False,
                   num_devices=NCORES)

    def din(name, shape, dt=F32):
        return nc.dram_tensor(name, shape, dt, kind="ExternalInput").ap()

    qtin = din("qtin", [P, 2, HT, BL], BF16)
    eptin = din("eptin", [P, 2, HT, NL], BF16)
    sktin = din("sktin", [P, 2, HT, ML], BF16)
    wq_t = din("wq_t", [HT, P, 2, HT, P], BF16)
    wek_t = din("wek_t", [HT, P, 2, HT, P], BF16)
    wsq_t = din("wsq_t", [HT, P, 2, HT, P], BF16)
    wsk_t = din("wsk_t", [HT, P, 2, HT, P], BF16)
    wev_t = din("wev_t", [4, P, HT, 512], BF16)
    weo_t = din("weo_t", [4, P, HT, 512], BF16)
    wso_t = din("wso_t", [4, P, HT, 512], BF16)
    wro_t = din("wro_t", [4, P, HT, 512], BF16)
    ep_store_b = din("ep_store_b", [N, H], BF16)
    sem_values_b = din("sem_values_b", [M, H], BF16)
    wsT_b = din("wsT_b", [P, HT, S], BF16)
    work_b = din("work_b", [S, H], BF16)
    gw1_b = din("gw1_b", [P, HT, 64], BF16)
    gw2_b = din("gw2_b", [64, 3], BF16)
    ep_imp = din("ep_imp", [N])
    ep_ts = din("ep_ts", [N])
    ep_imp_s = din("ep_imp_s", [NL])
    ep_ts_s = din("ep_ts_s", [NL])
    gate_b1 = din("gate_b1", [64])
    gate_b2 = din("gate_b2", [3])
    ln_gamma = din("ln_gamma", [H])
    ln_beta = din("ln_beta", [H])
    key_base = din("key_base", [1])

    out_s = nc.dram_tensor("out_s", [BL, H], F32, kind="ExternalOutput").ap()

    with tile.TileContext(nc) as tc:
        with (
            tc.tile_pool(name="cst", bufs=1) as cst,
            tc.tile_pool(name="rows", bufs=2) as rows,
            tc.tile_pool(name="sq", bufs=2) as sqp,
            tc.tile_pool(name="simc", bufs=2) as simcp,
            tc.tile_pool(name="tiny", bufs=2) as tiny,
            tc.tile_pool(name="gath", bufs=2) as gath,
            tc.tile_pool(name="ps_mm", bufs=3, space="PSUM") as ps_mm,
            tc.tile_pool(name="ps_tr", bufs=1, space="PSUM") as ps_tr,
            tc.tile_pool(name="ps_sml", bufs=2, space="PSUM") as ps_sml,
            tc.tile_pool(name="dram", bufs=1, space="DRAM") as dram,
        ):
            ident = cst.tile([P, P], F32)
            make_identity(nc, ident[:])
            ident_b = cst.tile([P, P], BF16)
            nc.vector.tensor_copy(ident_b[:], ident[:])
            ones_col = cst.tile([P, 1], F32)
            nc.vector.memset(ones_col[:], 1.0)

            ag_ek_in = dram.tile([2 * H, NL], BF16, name="ag_ek_in")
            ag_ek_out = dram.tile([NCORES * 2 * H, NL], BF16,
                                  addr_space="Shared", name="ag_ek_out")
            ag_qs_in = dram.tile([2 * H, BL], BF16, name="ag_qs_in")
            ag_qs_out = dram.tile([NCORES * 2 * H, BL], BF16,
                                  addr_space="Shared", name="ag_qs_out")
            ks_dram = dram.tile([2 * H, ML], BF16, name="ks_dram")
            cand_in = dram.tile([B, 8], F32, name="cand_in")
            cand_out = dram.tile([B, 8], F32, name="cand_out")
            bounce = dram.tile([2, BL], F32, name="bounce")

            # ---------- helpers ----------
            def load_wcol(pool, w_ap, j):
                t = pool.tile([P, 2, HT, P], BF16, tag="wcol", name="wcol",
                              bufs=2)
                nc.sync.dma_start(t[:], w_ap[j])
                return t

            def mm3(ps, stat, mov, s_sl=slice(None), m_sl=slice(None)):
                """ps = sum_hi [ Sh.T Mh + Sl.T Mh + Sh.T Ml ]."""
                for hi in range(HT):
                    sh = stat[:, 0, hi, s_sl]
                    sl = stat[:, 1, hi, s_sl]
                    mh = mov[:, 0, hi, m_sl]
                    ml = mov[:, 1, hi, m_sl]
                    nc.tensor.matmul(ps, sh, mh, start=(hi == 0), stop=False)
                    nc.tensor.matmul(ps, sl, mh, start=False, stop=False)
                    nc.tensor.matmul(ps, sh, ml, start=False,
                                     stop=(hi == HT - 1))

            def finish_inv_row(psn, width, extra_row=None):
                row = rows.tile([1, 512], F32, tag="nrow", name="nrow")
                nc.vector.tensor_copy(row[:1, :width], psn[:1, :width])
                nc.scalar.sqrt(row[:1, :width], row[:1, :width])
                nc.vector.tensor_scalar_max(row[:1, :width], row[:1, :width],
                                            1e-12)
                nc.vector.reciprocal(row[:1, :width], row[:1, :width])
                if extra_row is not None:
                    nc.vector.tensor_mul(row[:1, :width], row[:1, :width],
                                         extra_row)
                return row

            def bcast_row_dram(dram_row, width, name):
                row = rows.tile([1, width], F32, tag="crow", name="crow")
                nc.sync.dma_start(row[:1, :], dram_row)
                t = cst.tile([P, width], F32, name=name)
                nc.gpsimd.partition_broadcast(t[:, :], row[:1, :])
                return t

            # =================================================================
            # Phase W: episodic recency/importance weights
            # =================================================================
            def rec_weight(imp_ap, ts_ap, shape, tagb):
                impt = rows.tile(shape, F32, tag=tagb + "i", name="impt")
                tst = rows.tile(shape, F32, tag=tagb + "t", name="tst")
                nc.sync.dma_start(impt[:shape[0], :], imp_ap)
                nc.sync.dma_start(tst[:shape[0], :], ts_ap)
                s = tst[:shape[0], :]
                nc.scalar.activation(s, s, AF.Copy, bias=0.0, scale=-1.0)
                nc.vector.tensor_scalar_add(s, s, 1.0)
                nc.scalar.activation(s, s, AF.Abs)
                nc.scalar.activation(s, s, AF.Exp, scale=-RECENCY)
                si = impt[:shape[0], :]
                nc.vector.tensor_scalar_add(si, si, 1.0)
                nc.vector.tensor_mul(si, si, s)
                return impt

            wfull = rec_weight(ep_imp.rearrange("(p c) -> p c", p=P),
                               ep_ts.rearrange("(p c) -> p c", p=P),
                               [P, N // P], "wf")
            wpart = rows.tile([P, 1], F32, tag="wpart", name="wpart")
            nc.vector.reduce_sum(wpart[:, :], wfull[:, :], axis=AXL.X)
            pssum = ps_sml.tile([1, 512], F32, tag="nrm", name="wsps", bufs=1)
            nc.tensor.matmul(pssum[:1, :1], ones_col[:], wpart[:, :],
                             start=True, stop=True)
            wsum = rows.tile([1, 1], F32, tag="wsum", name="wsum")
            nc.vector.tensor_copy(wsum[:1, :], pssum[:1, :1])
            nc.vector.tensor_scalar_add(wsum[:1, :], wsum[:1, :], 1e-8)
            nc.vector.reciprocal(wsum[:1, :], wsum[:1, :])
            wloc = rec_weight(ep_imp_s[None, :], ep_ts_s[None, :], [1, NL],
                              "wl")
            nc.vector.tensor_scalar(wloc[:1, :], wloc[:1, :], wsum[:1, :1],
                                    None, op0=ALU.mult)

            # =================================================================
            # Phase EK: project episodic keys, scale by w/||k||, split, AG
            # =================================================================
            with tc.tile_pool(name="ph_ek", bufs=1) as ph_ek:
                ept = ph_ek.tile([P, 2, HT, NL], BF16, tag="ept", name="ept")
                nc.sync.dma_start(ept[:], eptin)
                ekf = ph_ek.tile([P, HT, NL], F32, tag="ekf", name="ekf")
                psn_ek = ps_sml.tile([1, 512], F32, tag="nrm", name="psn_ek",
                                     bufs=1)
                for j in range(HT):
                    wc = load_wcol(ph_ek, wek_t, j)
                    ps = ps_mm.tile([P, 512], F32, tag="mm", name="ps_ek")
                    mm3(ps[:], wc, ept)
                    nc.vector.tensor_copy(ekf[:, j, :], ps[:])
                    sq = sqp.tile([P, 512], F32, tag="sq", name="sq_ek")
                    nc.scalar.square(sq[:, :], ps[:])
                    nc.tensor.matmul(psn_ek[:1, :], ones_col[:], sq[:, :],
                                     start=(j == 0), stop=(j == HT - 1))
                inv_ek = finish_inv_row(psn_ek, NL, extra_row=wloc[:1, :])
                bc_ek = sqp.tile([P, 512], F32, tag="sq", name="bc_ek")
                nc.gpsimd.partition_broadcast(bc_ek[:, :], inv_ek[:1, :])
                ek_hl = ph_ek.tile([P, 2, HT, NL], BF16, tag="ekhl",
                                   name="ek_hl")
                for j in range(HT):
                    t = sqp.tile([P, 512], F32, tag="sq", name="t_ek")
                    nc.vector.tensor_mul(t[:, :], ekf[:, j, :], bc_ek[:, :])
                    nc.scalar.activation(ek_hl[:, 0, j, :], t[:, :], AF.Copy)
                    nc.vector.tensor_sub(ek_hl[:, 1, j, :], t[:, :],
                                         ek_hl[:, 0, j, :])
                nc.sync.dma_start(
                    ag_ek_in[0:H, :].rearrange("(hi p) c -> p hi c", p=P),
                    ek_hl[:, 0, :, :])
                nc.sync.dma_start(
                    ag_ek_in[H:2 * H, :].rearrange("(hi p) c -> p hi c", p=P),
                    ek_hl[:, 1, :, :])
                nc.gpsimd.collective_compute(
                    "AllGather", ALU.bypass,
                    replica_groups=[list(range(NCORES))],
                    ins=[ag_ek_in.opt()], outs=[ag_ek_out.opt()])

            with tc.tile_pool(name="ph_acc", bufs=1) as ph_acc:
                with tc.tile_pool(name="ph_qhl", bufs=1) as ph_qhl:
                    # =========================================================
                    # Phase Q: project queries, split (unscaled), norms
                    # =========================================================
                    q_hl = ph_qhl.tile([P, 2, HT, BL], BF16, tag="qhl",
                                       name="q_hl")
                    with tc.tile_pool(name="ph_qt", bufs=1) as ph_qt:
                        qt = ph_qt.tile([P, 2, HT, BL], BF16, tag="qt",
                                        name="qt")
                        nc.sync.dma_start(qt[:], qtin)
                        psn_q = ps_sml.tile([1, 512], F32, tag="nrm",
                                            name="psn_q", bufs=1)
                        for j in range(HT):
                            wc = load_wcol(ph_qhl, wq_t, j)
                            ps = ps_mm.tile([P, 512], F32, tag="mm",
                                            name="ps_q")
                            mm3(ps[:], wc, qt)
                            nc.scalar.activation(q_hl[:, 0, j, :], ps[:],
                                                 AF.Copy)
                            nc.vector.tensor_sub(q_hl[:, 1, j, :], ps[:],
                                                 q_hl[:, 0, j, :])
                            sq = sqp.tile([P, 512], F32, tag="sq", name="sq_q")
                            nc.scalar.square(sq[:, :], ps[:])
                            nc.tensor.matmul(psn_q[:1, :], ones_col[:],
                                             sq[:, :], start=(j == 0),
                                             stop=(j == HT - 1))
                        inv_q = finish_inv_row(psn_q, BL)
                        nc.sync.dma_start(bounce[0:1, :], inv_q[:1, :])

                    # =========================================================
                    # Phase QS: semantic query projection (unscaled)
                    # =========================================================
                    with tc.tile_pool(name="ph_qs", bufs=1) as ph_qs:
                        qs_hl = ph_qs.tile([P, 2, HT, BL], BF16, tag="qshl",
                                           name="qs_hl")
                        psn_qs = ps_sml.tile([1, 512], F32, tag="nrm",
                                             name="psn_qs", bufs=1)
                        for j in range(HT):
                            wc = load_wcol(ph_qhl, wsq_t, j)
                            ps = ps_mm.tile([P, 512], F32, tag="mm",
                                            name="ps_qs")
                            mm3(ps[:], wc, q_hl)
                            nc.scalar.activation(qs_hl[:, 0, j, :], ps[:],
                                                 AF.Copy)
                            nc.vector.tensor_sub(qs_hl[:, 1, j, :], ps[:],
                                                 qs_hl[:, 0, j, :])
                            sq = sqp.tile([P, 512], F32, tag="sq",
                                          name="sq_qs")
                            nc.scalar.square(sq[:, :], ps[:])
                            nc.tensor.matmul(psn_qs[:1, :], ones_col[:],
                                             sq[:, :], start=(j == 0),
                                             stop=(j == HT - 1))
                        inv_qs = finish_inv_row(psn_qs, BL)
                        nc.sync.dma_start(bounce[1:2, :], inv_qs[:1, :])
                        nc.sync.dma_start(
                            ag_qs_in[0:H, :].rearrange("(hi p) c -> p hi c",
                                                       p=P),
                            qs_hl[:, 0, :, :])
                        nc.sync.dma_start(
                            ag_qs_in[H:2 * H, :].rearrange(
                                "(hi p) c -> p hi c", p=P),
                            qs_hl[:, 1, :, :])
                        nc.gpsimd.collective_compute(
                            "AllGather", ALU.bypass,
                            replica_groups=[list(range(NCORES))],
                            ins=[ag_qs_in.opt()], outs=[ag_qs_out.opt()])

                    invq_p = cst.tile([P, NBT], F32, name="invq_p")
                    invqs_p = cst.tile([P, NBT], F32, name="invqs_p")
                    nc.sync.dma_start(
                        invq_p[:, :],
                        bounce[0:1, :].rearrange("o (t p) -> (o p) t", p=P))
                    nc.sync.dma_start(
                        invqs_p[:, :],
                        bounce[1:2, :].rearrange("o (t p) -> (o p) t", p=P))

                    # --- work attention + gate precompute ---
                    wsT = cst.tile([P, HT, S], BF16, name="wsT")
                    nc.sync.dma_start(wsT[:], wsT_b)
                    gw1 = cst.tile([P, HT, 64], BF16, name="gw1")
                    nc.sync.dma_start(gw1[:], gw1_b)
                    gw2 = cst.tile([64, 3], BF16, name="gw2")
                    nc.sync.dma_start(gw2[:, :], gw2_b)
                    b1bc = bcast_row_dram(gate_b1[None, :], 64, "b1bc")
                    b2bc = bcast_row_dram(gate_b2[None, :], 3, "b2bc")
                    kb_bc = bcast_row_dram(key_base[None, :], 1, "kb_bc")

                    inv_sqrt_h = 1.0 / math.sqrt(H)
                    ewT_pre = []
                    gw_pre = []
                    for bt in range(NBT):
                        qsl = slice(bt * P, (bt + 1) * P)
                        psw = ps_sml.tile([P, S], F32, tag="sml", name="pswk", bufs=1)
                        for hi in range(HT):
                            nc.tensor.matmul(
                                psw[:, :S], q_hl[:, 0, hi, qsl], wsT[:, hi, :],
                                start=(hi == 0), stop=(hi == HT - 1))
                        wmax = tiny.tile([P, 1], F32, tag="c1", name="wmax")
                        nc.vector.reduce_max(wmax[:, :], psw[:, :S],
                                             axis=AXL.X)
                        nc.vector.tensor_scalar_mul(wmax[:, :], wmax[:, :],
                                                    -inv_sqrt_h)
                        ew = tiny.tile([P, S], F32, tag="c64", name="ew")
                        nc.scalar.activation(ew[:, :], psw[:, :S], AF.Exp,
                                             bias=wmax[:, :1],
                                             scale=inv_sqrt_h)
                        zw = tiny.tile([P, 1], F32, tag="c1", name="zw")
                        nc.vector.reduce_sum(zw[:, :], ew[:, :], axis=AXL.X)
                        nc.vector.reciprocal(zw[:, :], zw[:, :])
                        nc.vector.tensor_scalar(ew[:, :], ew[:, :],
                                                zw[:, :1], None, op0=ALU.mult)
                        pset = ps_tr.tile([S, P], F32, tag="tr", name="ewtp")
                        nc.tensor.transpose(out=pset[:S, :], in_=ew[:, :],
                                            identity=ident[:])
                        ewT = cst.tile([S, P], BF16, name=f"ewT{bt}")
                        nc.vector.tensor_copy(ewT[:, :], pset[:S, :])
                        ewT_pre.append(ewT)

                        psg = ps_sml.tile([P, 64], F32, tag="sml", name="psg", bufs=1)
                        for hi in range(HT):
                            nc.tensor.matmul(
                                psg[:, :64], q_hl[:, 0, hi, qsl],
                                gw1[:, hi, :],
                                start=(hi == 0), stop=(hi == HT - 1))
                        hid = tiny.tile([P, 64], F32, tag="c64", name="hid")
                        nc.vector.tensor_add(hid[:, :], psg[:, :64],
                                             b1bc[:, :])
                        nc.scalar.activation(hid[:, :], hid[:, :], AF.Silu)
                        psht = ps_tr.tile([64, P], F32, tag="tr", name="hidtp")
                        nc.tensor.transpose(out=psht[:64, :], in_=hid[:, :],
                                            identity=ident[:])
                        hidT = tiny.tile([64, P], BF16, tag="c128",
                                         name="hidT")
                        nc.vector.tensor_copy(hidT[:, :], psht[:64, :])
                        psg2 = ps_sml.tile([P, 3], F32, tag="sml", name="psg2", bufs=1)
                        nc.tensor.matmul(psg2[:, :3], hidT[:, :], gw2[:, :],
                                         start=True, stop=True)
                        gl = cst.tile([P, 3], F32, name=f"gl{bt}")
                        nc.vector.tensor_add(gl[:, :], psg2[:, :3], b2bc[:, :])
                        gmax = tiny.tile([P, 1], F32, tag="c1", name="gmax")
                        nc.vector.reduce_max(gmax[:, :], gl[:, :], axis=AXL.X)
                        nc.vector.tensor_scalar_mul(gmax[:, :], gmax[:, :],
                                                    -1.0)
                        nc.scalar.activation(gl[:, :], gl[:, :], AF.Exp,
                                             bias=gmax[:, :1])
                        gz = tiny.tile([P, 1], F32, tag="c1", name="gz")
                        nc.vector.reduce_sum(gz[:, :], gl[:, :], axis=AXL.X)
                        nc.vector.reciprocal(gz[:, :], gz[:, :])
                        nc.vector.tensor_scalar(gl[:, :], gl[:, :],
                                                gz[:, :1], None, op0=ALU.mult)
                        gw_pre.append(gl)

                    # =========================================================
                    # Phase KS: project semantic keys, split -> DRAM; norms
                    # =========================================================
                    bc_ks = [cst.tile([P, 512], F32, name=f"bc_ks{kc}")
                             for kc in range(4)]
                    with tc.tile_pool(name="ph_ks", bufs=1) as ph_ks:
                        for mc in range(ML // 512):
                            msl = slice(mc * 512, (mc + 1) * 512)
                            skt = ph_ks.tile([P, 2, HT, 512], BF16, tag="skt",
                                             name="skt")
                            nc.sync.dma_start(skt[:],
                                              sktin[:, :, :, msl])
                            psn = ps_sml.tile([1, 512], F32, tag="nrm",
                                              name="psn_ks", bufs=1)
                            for j in range(HT):
                                wc = load_wcol(ph_qhl, wsk_t, j)
                                ps = ps_mm.tile([P, 512], F32, tag="mm",
                                                name="ps_ks")
                                mm3(ps[:], wc, skt)
                                st = sqp.tile([P, 2, 512], BF16, tag="ksst",
                                              name="ksst")
                                nc.scalar.activation(st[:, 0, :], ps[:],
                                                     AF.Copy)
                                nc.vector.tensor_sub(st[:, 1, :], ps[:],
                                                     st[:, 0, :])
                                nc.sync.dma_start(
                                    ks_dram[j * P:(j + 1) * P, msl],
                                    st[:, 0, :])
                                nc.sync.dma_start(
                                    ks_dram[H + j * P:H + (j + 1) * P, msl],
                                    st[:, 1, :])
                                sq = sqp.tile([P, 512], F32, tag="sq",
                                              name="sq_ks")
                                nc.scalar.square(sq[:, :], ps[:])
                                nc.tensor.matmul(psn[:1, :], ones_col[:],
                                                 sq[:, :], start=(j == 0),
                                                 stop=(j == HT - 1))
                            inv = finish_inv_row(psn, 512)
                            nc.gpsimd.partition_broadcast(bc_ks[mc][:, :],
                                                          inv[:1, :512])

                    # =========================================================
                    # Phase SIM-E: own queries x all episodic keys
                    # =========================================================
                    cand_v_e = [cst.tile([P, 128], F32, name=f"cve{bt}")
                                for bt in range(NBT)]
                    cand_i_e = [cst.tile([P, 128], F32, name=f"cie{bt}")
                                for bt in range(NBT)]
                    with tc.tile_pool(name="ph_se", bufs=2) as ph_se:
                        for slab in range(NCORES):
                            base = slab * 2 * H
                            for khalf in range(2):
                                csl = slice(khalf * 256, (khalf + 1) * 256)
                                ekg = ph_se.tile([P, 2, HT, 256], BF16,
                                                 tag="ekg", name="ekg")
                                nc.sync.dma_start(
                                    ekg[:],
                                    ag_ek_out[base:base + 2 * H,
                                              csl].rearrange(
                                        "(s hi p) c -> p s hi c", p=P, s=2))
                                cid = 2 * slab + khalf
                                for bt in range(NBT):
                                    qsl = slice(bt * P, (bt + 1) * P)
                                    ps = ps_mm.tile([P, 512], F32, tag="mm",
                                                    name="ps_se")
                                    mm3(ps[:, :256], q_hl, ekg, s_sl=qsl)
                                    sc = simcp.tile([P, 256], F32, tag="sime",
                                                    name="sc_e")
                                    nc.vector.tensor_copy(sc[:], ps[:, :256])
                                    mx = tiny.tile([P, 8], F32, tag="mx",
                                                   name="mx_e")
                                    mi = tiny.tile([P, 8], U32, tag="mi",
                                                   name="mi_e")
                                    nc.vector.max(out=mx[:], in_=sc[:])
                                    nc.vector.max_index(out=mi[:],
                                                        in_max=mx[:],
                                                        in_values=sc[:])
                                    nc.vector.tensor_copy(
                                        cand_v_e[bt][:,
                                                     cid * 8:(cid + 1) * 8],
                                        mx[:])
                                    mif = tiny.tile([P, 8], F32, tag="mif",
                                                    name="mif_e")
                                    nc.vector.tensor_copy(mif[:], mi[:])
                                    nc.vector.tensor_scalar_add(
                                        cand_i_e[bt][:,
                                                     cid * 8:(cid + 1) * 8],
                                        mif[:], float(cid * 256))

                    # --- episodic top-8 merge + gather + weighted sum ---
                    acc_e_b = [ph_acc.tile([P, H], BF16, tag=f"acce{bt}",
                                           name=f"acce{bt}")
                               for bt in range(NBT)]
                    for bt in range(NBT):
                        top8 = tiny.tile([P, 8], F32, tag="c8", name="top8")
                        nc.vector.max(out=top8[:], in_=cand_v_e[bt][:])
                        idxf = tiny.tile([P, 8], F32, tag="c8b", name="idxf")
                        eqm = simcp.tile([P, 128], F32, tag="eqm", name="eqm")
                        for kk in range(EP_K):
                            nc.vector.tensor_scalar(
                                eqm[:, :], cand_v_e[bt][:],
                                top8[:, kk:kk + 1], None, op0=ALU.is_equal)
                            nc.vector.tensor_tensor(
                                out=eqm[:, :], in0=eqm[:, :],
                                in1=cand_i_e[bt][:], op=ALU.mult)
                            nc.vector.reduce_sum(idxf[:, kk:kk + 1],
                                                 eqm[:, :], axis=AXL.X)
                        idxu = tiny.tile([P, 8], U32, tag="c8u", name="idxu")
                        nc.vector.tensor_copy(idxu[:, :], idxf[:, :])
                        sc8 = tiny.tile([P, 8], F32, tag="c8c", name="sc8")
                        nc.vector.tensor_scalar(
                            sc8[:, :], top8[:, :], invq_p[:, bt:bt + 1], None,
                            op0=ALU.mult)
                        negm = tiny.tile([P, 1], F32, tag="c1", name="negm")
                        nc.vector.tensor_scalar_mul(negm[:, :], sc8[:, 0:1],
                                                    -1.0)
                        nc.scalar.activation(sc8[:, :], sc8[:, :], AF.Exp,
                                             bias=negm[:, :1])
                        zs = tiny.tile([P, 1], F32, tag="c1", name="zs")
                        nc.vector.reduce_sum(zs[:, :], sc8[:, :], axis=AXL.X)
                        nc.vector.reciprocal(zs[:, :], zs[:, :])
                        nc.vector.tensor_scalar(zs[:, :], zs[:, :],
                                                gw_pre[bt][:, 1:2], None,
                                                op0=ALU.mult)
                        nc.vector.tensor_scalar(sc8[:, :], sc8[:, :],
                                                zs[:, :1], None, op0=ALU.mult)
                        acc = simcp.tile([P, H], F32, tag="acc", name="acc_e",
                                         bufs=1)
                        nc.vector.memset(acc[:, :], 0.0)
                        for kk in range(EP_K):
                            g = gath.tile([P, H], BF16, tag="g", name="g_e")
                            nc.gpsimd.indirect_dma_start(
                                out=g[:, :], out_offset=None, in_=ep_store_b,
                                in_offset=bass.IndirectOffsetOnAxis(
                                    ap=idxu[:, kk:kk + 1], axis=0))
                            nc.vector.scalar_tensor_tensor(
                                out=acc[:, :], in0=g[:, :],
                                scalar=sc8[:, kk:kk + 1],
                                in1=acc[:, :], op0=ALU.mult, op1=ALU.add)
                        nc.vector.tensor_copy(acc_e_b[bt][:, :], acc[:, :])

                # ==== ph_qhl closed: q_hl freed ====
                # =============================================================
                # Phase SIM-S: ALL queries x local semantic keys (kc-outer)
                # =============================================================
                with tc.tile_pool(name="ph_ss", bufs=1) as ph_ss:
                    cand_sv = ph_ss.tile([P, 32 * 32], F32, tag="csv",
                                         name="cand_sv")
                    cand_si = ph_ss.tile([P, 32 * 32], F32, tag="csi",
                                         name="cand_si")
                    for kc in range(4):
                        msl = slice(kc * 512, (kc + 1) * 512)
                        ksc = ph_ss.tile([P, 2, HT, 512], BF16, tag="ksc",
                                         name="ksc")
                        nc.sync.dma_start(
                            ksc[:],
                            ks_dram[:, msl].rearrange(
                                "(s hi p) c -> p s hi c", p=P, s=2))
                        for rq2 in range(16):
                            slabq = rq2 // 2
                            base = slabq * 2 * H
                            col0 = (rq2 % 2) * 256
                            qsg = ph_ss.tile([P, 2, HT, 256], BF16, tag="qsg",
                                             name="qsg", bufs=2)
                            nc.sync.dma_start(
                                qsg[:],
                                ag_qs_out[base:base + 2 * H,
                                          col0:col0 + 256].rearrange(
                                    "(s hi p) c -> p s hi c", p=P, s=2))
                            for rq in range(2):
                                rqt = rq2 * 2 + rq
                                qssl = slice(rq * P, (rq + 1) * P)
                                ps = ps_mm.tile([P, 512], F32, tag="mm",
                                                name="ps_ss")
                                mm3(ps[:], qsg, ksc, s_sl=qssl)
                                sc = simcp.tile([P, 512], F32, tag="scs",
                                                name="sc_s")
                                nc.vector.tensor_mul(sc[:, :], ps[:],
                                                     bc_ks[kc][:, :])
                                mx = tiny.tile([P, 8], F32, tag="mx",
                                               name="mx_s")
                                mi = tiny.tile([P, 8], U32, tag="mi",
                                               name="mi_s")
                                nc.vector.max(out=mx[:], in_=sc[:])
                                nc.vector.max_index(out=mi[:], in_max=mx[:],
                                                    in_values=sc[:])
                                wsl = slice(rqt * 32 + kc * 8,
                                            rqt * 32 + (kc + 1) * 8)
                                nc.vector.tensor_copy(cand_sv[:, wsl], mx[:])
                                mif = tiny.tile([P, 8], F32, tag="mif",
                                                name="mif_s")
                                nc.vector.tensor_copy(mif[:], mi[:])
                                nc.vector.tensor_scalar_add(
                                    cand_si[:, wsl], mif[:], float(kc * 512))

                    # local top-4 per query, global index, ship via a2a
                    for rqt in range(32):
                        wsl = slice(rqt * 32, (rqt + 1) * 32)
                        top8 = tiny.tile([P, 8], F32, tag="c8", name="top8l")
                        nc.vector.max(out=top8[:], in_=cand_sv[:, wsl])
                        idxf = tiny.tile([P, 8], F32, tag="c8b", name="idxfl")
                        eqm = simcp.tile([P, 32], F32, tag="eqs", name="eqml")
                        for kk in range(SEM_K):
                            nc.vector.tensor_scalar(
                                eqm[:, :], cand_sv[:, wsl],
                                top8[:, kk:kk + 1], None, op0=ALU.is_equal)
                            nc.vector.tensor_tensor(
                                out=eqm[:, :], in0=eqm[:, :],
                                in1=cand_si[:, wsl], op=ALU.mult)
                            nc.vector.reduce_sum(idxf[:, kk:kk + 1],
                                                 eqm[:, :], axis=AXL.X)
                        p4 = tiny.tile([P, 8], F32, tag="p4", name="p4")
                        nc.vector.tensor_copy(p4[:, 0:4], top8[:, 0:4])
                        nc.vector.tensor_scalar(
                            p4[:, 4:8], idxf[:, 0:4], kb_bc[:, 0:1], None,
                            op0=ALU.add)
                        nc.sync.dma_start(cand_in[rqt * P:(rqt + 1) * P, :],
                                          p4[:, :])
                nc.gpsimd.collective_compute(
                    "AllToAll", ALU.bypass,
                    replica_groups=[list(range(NCORES))],
                    ins=[cand_in.opt()], outs=[cand_out.opt()])

                # =============================================================
                # Phase FINAL
                # =============================================================
                with tc.tile_pool(name="fin", bufs=1) as fin:
                    def transpose_b(src_b, dst):
                        for hi in range(HT):
                            pst = ps_tr.tile([P, P], BF16, tag="trb16",
                                             name="trp", bufs=2)
                            nc.tensor.transpose(
                                out=pst[:], in_=src_b[:, hi * P:(hi + 1) * P],
                                identity=ident_b[:])
                            nc.vector.tensor_copy(dst[:, hi, :], pst[:])

                    def val_stage(w_ap, accT_list, out_tiles, mode,
                                  gscale=None):
                        for jc in range(4):
                            wv = fin.tile([P, HT, 512], BF16, tag="wv",
                                          name="wv", bufs=2)
                            nc.sync.dma_start(wv[:], w_ap[jc])
                            jsl = slice(jc * 512, (jc + 1) * 512)
                            for bt in range(NBT):
                                ps = ps_mm.tile([P, 512], F32, tag="mm",
                                                name="ps_v")
                                for hi in range(HT):
                                    nc.tensor.matmul(
                                        ps[:], accT_list[bt][:, hi, :],
                                        wv[:, hi, :], start=(hi == 0),
                                        stop=(hi == HT - 1))
                                if mode == "set":
                                    nc.vector.tensor_copy(
                                        out_tiles[bt][:, jsl], ps[:])
                                else:
                                    nc.vector.tensor_add(
                                        out_tiles[bt][:, jsl],
                                        out_tiles[bt][:, jsl], ps[:])

                    # e chain: tmp_e = acc_e @ W_ev
                    accT = [fin.tile([P, HT, P], BF16, tag="accT",
                                     name=f"accT{bt}", bufs=4)
                            for bt in range(NBT)]
                    for bt in range(NBT):
                        transpose_b(acc_e_b[bt], accT[bt])
                    tmp_e = [fin.tile([P, H], BF16, tag="t16",
                                      name=f"tmpe{bt}", bufs=4)
                             for bt in range(NBT)]
                    val_stage(wev_t, accT, tmp_e, "set")
                    accT2 = [fin.tile([P, HT, P], BF16, tag="accT",
                                      name=f"accT2{bt}", bufs=4)
                             for bt in range(NBT)]
                    for bt in range(NBT):
                        transpose_b(tmp_e[bt], accT2[bt])

                    # bl = gl0 * w_out
                    bl = [fin.tile([P, H], F32, tag="f32b", name=f"bl{bt}",
                                   bufs=4)
                          for bt in range(NBT)]
                    for jc in range(4):
                        wvw = fin.tile([S, 512], BF16, tag="wvw", name="wvw",
                                       bufs=2)
                        nc.sync.dma_start(wvw[:S, :],
                                          work_b[:, jc * 512:(jc + 1) * 512])
                        jsl = slice(jc * 512, (jc + 1) * 512)
                        for bt in range(NBT):
                            ps = ps_mm.tile([P, 512], F32, tag="mm",
                                            name="ps_w")
                            nc.tensor.matmul(ps[:], ewT_pre[bt][:, :],
                                             wvw[:S, :], start=True,
                                             stop=True)
                            nc.vector.tensor_scalar(
                                bl[bt][:, jsl], ps[:], gw_pre[bt][:, 0:1],
                                None, op0=ALU.mult)

                    # bl += tmp_e @ W_eo
                    val_stage(weo_t, accT2, bl, "add")

                    # --- semantic merge + gather (after AllToAll) ---
                    acc_s_b = [ph_acc.tile([P, H], BF16, tag=f"accs{bt}",
                                           name=f"accs{bt}")
                               for bt in range(NBT)]
                    for bt in range(NBT):
                        c32v = simcp.tile([P, 32], F32, tag="eqs",
                                          name="c32v")
                        c32i = simcp.tile([P, 32], F32, tag="eqs2",
                                          name="c32i")
                        for r in range(NCORES):
                            c8 = tiny.tile([P, 8], F32, tag="p4", name="c8in")
                            nc.sync.dma_start(
                                c8[:, :],
                                cand_out[r * BL + bt * P:
                                         r * BL + (bt + 1) * P, :])
                            nc.vector.tensor_copy(c32v[:, r * 4:(r + 1) * 4],
                                                  c8[:, 0:4])
                            nc.vector.tensor_copy(c32i[:, r * 4:(r + 1) * 4],
                                                  c8[:, 4:8])
                        top8 = tiny.tile([P, 8], F32, tag="c8", name="top8s")
                        nc.vector.max(out=top8[:], in_=c32v[:])
                        idxf = tiny.tile([P, 8], F32, tag="c8b", name="idxfs")
                        eqs = simcp.tile([P, 32], F32, tag="eqs3", name="eqs")
                        for kk in range(SEM_K):
                            nc.vector.tensor_scalar(
                                eqs[:, :], c32v[:, :], top8[:, kk:kk + 1],
                                None, op0=ALU.is_equal)
                            nc.vector.tensor_tensor(out=eqs[:, :],
                                                    in0=eqs[:, :],
                                                    in1=c32i[:, :],
                                                    op=ALU.mult)
                            nc.vector.reduce_sum(idxf[:, kk:kk + 1],
                                                 eqs[:, :], axis=AXL.X)
                        idxu = tiny.tile([P, 8], U32, tag="c8u", name="idxus")
                        nc.vector.tensor_copy(idxu[:, 0:4], idxf[:, 0:4])
                        sc4 = tiny.tile([P, 4], F32, tag="c4", name="sc4")
                        nc.vector.tensor_scalar(
                            sc4[:, :], top8[:, 0:4], invqs_p[:, bt:bt + 1],
                            None, op0=ALU.mult)
                        negm = tiny.tile([P, 1], F32, tag="c1", name="negms")
                        nc.vector.tensor_scalar_mul(negm[:, :], sc4[:, 0:1],
                                                    -1.0)
                        nc.scalar.activation(sc4[:, :], sc4[:, :], AF.Exp,
                                             bias=negm[:, :1])
                        zs = tiny.tile([P, 1], F32, tag="c1", name="zss")
                        nc.vector.reduce_sum(zs[:, :], sc4[:, :], axis=AXL.X)
                        nc.vector.reciprocal(zs[:, :], zs[:, :])
                        nc.vector.tensor_scalar(zs[:, :], zs[:, :],
                                                gw_pre[bt][:, 2:3], None,
                                                op0=ALU.mult)
                        nc.vector.tensor_scalar(sc4[:, :], sc4[:, :],
                                                zs[:, :1], None, op0=ALU.mult)
                        acc = simcp.tile([P, H], F32, tag="acc", name="acc_s",
                                         bufs=1)
                        nc.vector.memset(acc[:, :], 0.0)
                        for kk in range(SEM_K):
                            g = gath.tile([P, H], BF16, tag="g", name="g_s")
                            nc.gpsimd.indirect_dma_start(
                                out=g[:, :], out_offset=None,
                                in_=sem_values_b,
                                in_offset=bass.IndirectOffsetOnAxis(
                                    ap=idxu[:, kk:kk + 1], axis=0))
                            nc.vector.scalar_tensor_tensor(
                                out=acc[:, :], in0=g[:, :],
                                scalar=sc4[:, kk:kk + 1],
                                in1=acc[:, :], op0=ALU.mult, op1=ALU.add)
                        nc.vector.tensor_copy(acc_s_b[bt][:, :], acc[:, :])

                    # bl += acc_s @ W_so
                    accT_s = [fin.tile([P, HT, P], BF16, tag="accT",
                                       name=f"accTs{bt}", bufs=4)
                              for bt in range(NBT)]
                    for bt in range(NBT):
                        transpose_b(acc_s_b[bt], accT_s[bt])
                    val_stage(wso_t, accT_s, bl, "add")

                    # xo = bl @ W_ro; out = LN(xo)*gamma+beta
                    blb = [fin.tile([P, H], BF16, tag="t16", name=f"blb{bt}",
                                    bufs=4)
                           for bt in range(NBT)]
                    for bt in range(NBT):
                        nc.vector.tensor_copy(blb[bt][:, :], bl[bt][:, :])
                    accT_bl = [fin.tile([P, HT, P], BF16, tag="accT",
                                        name=f"accTb{bt}", bufs=4)
                               for bt in range(NBT)]
                    for bt in range(NBT):
                        transpose_b(blb[bt], accT_bl[bt])
                    xo = [fin.tile([P, H], F32, tag="f32b", name=f"xo{bt}",
                                   bufs=4)
                          for bt in range(NBT)]
                    val_stage(wro_t, accT_bl, xo, "set")

                    for bt in range(NBT):
                        x = xo[bt]
                        mu = tiny.tile([P, 1], F32, tag="c1", name="mu")
                        nc.vector.reduce_sum(mu[:, :], x[:, :], axis=AXL.X)
                        nc.vector.tensor_scalar_mul(mu[:, :], mu[:, :],
                                                    -1.0 / H)
                        nc.vector.tensor_scalar(x[:, :], x[:, :], mu[:, :1],
                                                None, op0=ALU.add)
                        sqx = simcp.tile([P, H], F32, tag="acc", name="sqx",
                                         bufs=1)
                        vs = tiny.tile([P, 1], F32, tag="c1", name="vs")
                        nc.scalar.activation(sqx[:, :], x[:, :], AF.Square,
                                             accum_out=vs[:, :1])
                        nc.vector.tensor_scalar_mul(vs[:, :], vs[:, :],
                                                    1.0 / H)
                        nc.vector.tensor_scalar_add(vs[:, :], vs[:, :],
                                                    LN_EPS)
                        nc.scalar.sqrt(vs[:, :], vs[:, :])
                        nc.vector.reciprocal(vs[:, :], vs[:, :])
                        nc.vector.tensor_scalar(x[:, :], x[:, :], vs[:, :1],
                                                None, op0=ALU.mult)
                        for jc in range(4):
                            jsl = slice(jc * 512, (jc + 1) * 512)
                            gbch = sqp.tile([P, 512], F32, tag="sq",
                                            name="gbch")
                            grow = rows.tile([1, 512], F32, tag="crow",
                                             name="grow")
                            nc.sync.dma_start(grow[:1, :],
                                              ln_gamma[None, jsl])
                            nc.gpsimd.partition_broadcast(gbch[:, :],
                                                          grow[:1, :])
                            nc.vector.tensor_mul(x[:, jsl], x[:, jsl],
                                                 gbch[:, :])
                            bbch = sqp.tile([P, 512], F32, tag="sq",
                                            name="bbch")
                            brow = rows.tile([1, 512], F32, tag="crow",
                                             name="brow")
                            nc.sync.dma_start(brow[:1, :],
                                              ln_beta[None, jsl])
                            nc.gpsimd.partition_broadcast(bbch[:, :],
                                                          brow[:1, :])
                            nc.vector.tensor_add(x[:, jsl], x[:, jsl],
                                                 bbch[:, :])
                        nc.sync.dma_start(out_s[bt * P:(bt + 1) * P, :],
                                          x[:, :])

    nc.finalize()
    return nc


_NC_CACHE = None


def _bf16_split(x):
    h = x.astype(ml_dtypes.bfloat16)
    l = (x - h.astype(np.float32)).astype(ml_dtypes.bfloat16)
    return h, l


def _tile_sel_weight(w):
    """[H, H] f32 -> [j, p, 2, hi, 128] bf16 hi/lo tiled."""
    h, l = _bf16_split(w)
    out = np.empty((HT, P, 2, HT, P), dtype=ml_dtypes.bfloat16)
    hr = h.reshape(HT, P, HT, P)   # [hi, p, j, c]
    lr = l.reshape(HT, P, HT, P)
    out[:, :, 0] = hr.transpose(2, 1, 0, 3)
    out[:, :, 1] = lr.transpose(2, 1, 0, 3)
    return np.ascontiguousarray(out)


def _tile_val_weight(w):
    """[H, H] f32 -> [jc, p, hi, 512] bf16."""
    b = w.astype(ml_dtypes.bfloat16)
    r = b.reshape(HT, P, 4, 512)   # [hi, p, jc, c]
    return np.ascontiguousarray(r.transpose(2, 1, 0, 3))


def _split_T(x):
    """[R, H] f32 -> [p, 2, hi, R] bf16 (transposed hi/lo)."""
    h, l = _bf16_split(x)
    R = x.shape[0]
    out = np.empty((P, 2, HT, R), dtype=ml_dtypes.bfloat16)
    out[:, 0] = h.T.reshape(HT, P, R).transpose(1, 0, 2)
    out[:, 1] = l.T.reshape(HT, P, R).transpose(1, 0, 2)
    return np.ascontiguousarray(out)


def kernel(**inputs) -> np.ndarray:
    global _NC_CACHE
    if _NC_CACHE is None:
        _NC_CACHE = build()
    nc = _NC_CACHE

    f32 = lambda x: np.ascontiguousarray(np.asarray(x), dtype=np.float32)
    query = f32(inputs["query"])
    ep_store = f32(inputs["ep_store"])
    sem_keys = f32(inputs["sem_keys"])
    work_slots = f32(inputs["work_slots"])

    shared = {
        "wq_t": _tile_sel_weight(f32(inputs["W_query"])),
        "wek_t": _tile_sel_weight(f32(inputs["W_ek"])),
        "wsq_t": _tile_sel_weight(f32(inputs["W_sq"])),
        "wsk_t": _tile_sel_weight(f32(inputs["W_sk"])),
        "wev_t": _tile_val_weight(f32(inputs["W_ev"])),
        "weo_t": _tile_val_weight(f32(inputs["W_eo"])),
        "wso_t": _tile_val_weight(f32(inputs["W_so"])),
        "wro_t": _tile_val_weight(f32(inputs["W_ro"])),
        "ep_store_b": ep_store.astype(ml_dtypes.bfloat16),
        "sem_values_b": f32(inputs["sem_values"]).astype(ml_dtypes.bfloat16),
        "wsT_b": np.ascontiguousarray(
            work_slots.T.astype(ml_dtypes.bfloat16).reshape(HT, P, S)
            .transpose(1, 0, 2)),
        "work_b": work_slots.astype(ml_dtypes.bfloat16),
        "gw1_b": np.ascontiguousarray(
            f32(inputs["gate_W1"]).astype(ml_dtypes.bfloat16)
            .reshape(HT, P, 64).transpose(1, 0, 2)),
        "gw2_b": f32(inputs["gate_W2"]).astype(ml_dtypes.bfloat16),
        "ep_imp": f32(inputs["ep_importance"]),
        "ep_ts": f32(inputs["ep_timestamps"]),
        "gate_b1": f32(inputs["gate_b1"]),
        "gate_b2": f32(inputs["gate_b2"]),
        "ln_gamma": f32(inputs["ln_gamma"]),
        "ln_beta": f32(inputs["ln_beta"]),
    }

    in_maps = []
    for c in range(NCORES):
        m = dict(shared)
        m["qtin"] = _split_T(query[c * BL:(c + 1) * BL])
        m["eptin"] = _split_T(ep_store[c * NL:(c + 1) * NL])
        m["sktin"] = _split_T(sem_keys[c * ML:(c + 1) * ML])
        m["ep_imp_s"] = f32(inputs["ep_importance"][c * NL:(c + 1) * NL])
        m["ep_ts_s"] = f32(inputs["ep_timestamps"][c * NL:(c + 1) * NL])
        m["key_base"] = np.array([c * ML], dtype=np.float32)
        in_maps.append(m)

    res = run_bass_kernel_spmd(nc, in_maps, core_ids=list(range(NCORES)))
    return np.concatenate([res.results[c]["out_s"] for c in range(NCORES)],
                          axis=0)


# revision 14
# speedup vs baseline: 1.6089x; 1.0271x over previous
"""ONIMemoryHub kernel for 8 Trainium2 NeuronCores (Bass/Tile).

Strategy (v2):
- Selection path (projections feeding top-k similarity + the similarity
  matmuls) runs as 3-term bf16 hi/lo splits: x@W = xh@Wh + xl@Wh + xh@Wl,
  ~2^-19 relative accuracy at 3 PE cycles/row (vs 4 for fp32).
- Values path (W_ev/W_eo/W_so/W_ro, work/gate) runs in plain bf16.
- Episodic: keys projected/normalized/weighted on the owning core, packed
  hi/lo and AllGathered; each core scans all N keys for its own queries.
  Top-k attend gathers RAW ep_store rows (replicated input) and applies
  W_ev @ W_eo after the weighted sum (linearity) - no value AllGather.
- Semantic: keys stay sharded; query projections (qs) are AllGathered
  (hi/lo packed); each core scans ALL queries against its local keys and
  takes local top-4 per query; an AllToAll returns every core's candidates
  for the queries each core owns; exact merge + softmax + gather of raw
  sem_values happens on the query owner. Per-key 1/||ks|| is applied to sim
  rows pre-top-k; per-query 1/||qs|| post-merge (order-invariant).
- Host precomputes transposes and bf16 hi/lo splits of inputs/weights.

kernel(**inputs) takes FULL inputs and returns the FULL [4096, 2048] output.
"""
import math

import numpy as np
import ml_dtypes

import concourse.bass as bass
import concourse.mybir as mybir
import concourse.tile as tile
from concourse import bacc
from concourse.bass_utils import run_bass_kernel_spmd
from concourse.masks import make_identity

AF = mybir.ActivationFunctionType
AXL = mybir.AxisListType
ALU = mybir.AluOpType

NCORES = 8
B, H, N, M, S = 4096, 2048, 4096, 16384, 64
BL, NL, ML = B // NCORES, N // NCORES, M // NCORES   # 512, 512, 2048
P = 128
HT = H // P                                          # 16
NBT = BL // P                                        # 4
EP_K = 8
SEM_K = 4
LN_EPS = 1e-5
RECENCY = 0.01

F32 = mybir.dt.float32
BF16 = mybir.dt.bfloat16
U32 = mybir.dt.uint32


def build():
    nc = bacc.Bacc("TRN2", target_bir_lowering=False, debug=False,
                   num_devices=NCORES)

    def din(name, shape, dt=F32):
        return nc.dram_tensor(name, shape, dt, kind="ExternalInput").ap()

    qtin = din("qtin", [P, 2, HT, BL], BF16)
    eptin = din("eptin", [P, 2, HT, NL], BF16)
    sktin = din("sktin", [P, 2, HT, ML], BF16)
    wq_t = din("wq_t", [HT, P, 2, HT, P], BF16)
    wek_t = din("wek_t", [HT, P, 2, HT, P], BF16)
    wsq_t = din("wsq_t", [HT, P, 2, HT, P], BF16)
    wsk_t = din("wsk_t", [HT, P, 2, HT, P], BF16)
    wev_t = din("wev_t", [4, P, HT, 512], BF16)
    weo_t = din("weo_t", [4, P, HT, 512], BF16)
    wso_t = din("wso_t", [4, P, HT, 512], BF16)
    wro_t = din("wro_t", [4, P, HT, 512], BF16)
    ep_store_b = din("ep_store_b", [N, H], BF16)
    sem_values_b = din("sem_values_b", [M, H], BF16)
    wsT_b = din("wsT_b", [P, HT, S], BF16)
    work_b = din("work_b", [S, H], BF16)
    gw1_b = din("gw1_b", [P, HT, 64], BF16)
    gw2_b = din("gw2_b", [64, 3], BF16)
    ep_imp = din("ep_imp", [N])
    ep_ts = din("ep_ts", [N])
    ep_imp_s = din("ep_imp_s", [NL])
    ep_ts_s = din("ep_ts_s", [NL])
    gate_b1 = din("gate_b1", [64])
    gate_b2 = din("gate_b2", [3])
    ln_gamma = din("ln_gamma", [H])
    ln_beta = din("ln_beta", [H])
    key_base = din("key_base", [1])

    out_s = nc.dram_tensor("out_s", [BL, H], F32, kind="ExternalOutput").ap()

    with tile.TileContext(nc) as tc:
        with (
            tc.tile_pool(name="cst", bufs=1) as cst,
            tc.tile_pool(name="rows", bufs=2) as rows,
            tc.tile_pool(name="sq", bufs=2) as sqp,
            tc.tile_pool(name="simc", bufs=2) as simcp,
            tc.tile_pool(name="tiny", bufs=2) as tiny,
            tc.tile_pool(name="gath", bufs=2) as gath,
            tc.tile_pool(name="ps_mm", bufs=3, space="PSUM") as ps_mm,
            tc.tile_pool(name="ps_tr", bufs=1, space="PSUM") as ps_tr,
            tc.tile_pool(name="ps_sml", bufs=2, space="PSUM") as ps_sml,
            tc.tile_pool(name="dram", bufs=1, space="DRAM") as dram,
        ):
            ident = cst.tile([P, P], F32)
            make_identity(nc, ident[:])
            ident_b = cst.tile([P, P], BF16)
            nc.vector.tensor_copy(ident_b[:], ident[:])
            ones_col = cst.tile([P, 1], F32)
            nc.vector.memset(ones_col[:], 1.0)

            ag_ek_in = dram.tile([2 * H, NL], BF16, name="ag_ek_in")
            ag_ek_out = dram.tile([NCORES * 2 * H, NL], BF16,
                                  addr_space="Shared", name="ag_ek_out")
            ag_qs_in = dram.tile([2 * H, BL], BF16, name="ag_qs_in")
            ag_qs_out = dram.tile([NCORES * 2 * H, BL], BF16,
                                  addr_space="Shared", name="ag_qs_out")
            ks_dram = dram.tile([2 * H, ML], BF16, name="ks_dram")
            cand_in = dram.tile([B, 8], F32, name="cand_in")
            cand_out = dram.tile([B, 8], F32, name="cand_out")
            bounce = dram.tile([2, BL], F32, name="bounce")

            # ---------- helpers ----------
            def load_wcol(pool, w_ap, j):
                t = pool.tile([P, 2, HT, P], BF16, tag="wcol", name="wcol",
                              bufs=2)
                nc.sync.dma_start(t[:], w_ap[j])
                return t

            def mm3(ps, stat, mov, s_sl=slice(None), m_sl=slice(None)):
                """ps = sum_hi [ Sh.T Mh + Sl.T Mh + Sh.T Ml ]."""
                for hi in range(HT):
                    sh = stat[:, 0, hi, s_sl]
                    sl = stat[:, 1, hi, s_sl]
                    mh = mov[:, 0, hi, m_sl]
                    ml = mov[:, 1, hi, m_sl]
                    nc.tensor.matmul(ps, sh, mh, start=(hi == 0), stop=False)
                    nc.tensor.matmul(ps, sl, mh, start=False, stop=False)
                    nc.tensor.matmul(ps, sh, ml, start=False,
                                     stop=(hi == HT - 1))

            def finish_inv_row(psn, width, extra_row=None):
                row = rows.tile([1, 512], F32, tag="nrow", name="nrow")
                nc.vector.tensor_copy(row[:1, :width], psn[:1, :width])
                nc.scalar.sqrt(row[:1, :width], row[:1, :width])
                nc.vector.tensor_scalar_max(row[:1, :width], row[:1, :width],
                                            1e-12)
                nc.vector.reciprocal(row[:1, :width], row[:1, :width])
                if extra_row is not None:
                    nc.vector.tensor_mul(row[:1, :width], row[:1, :width],
                                         extra_row)
                return row

            def bcast_row_dram(dram_row, width, name):
                row = rows.tile([1, width], F32, tag="crow", name="crow")
                nc.sync.dma_start(row[:1, :], dram_row)
                t = cst.tile([P, width], F32, name=name)
                nc.gpsimd.partition_broadcast(t[:, :], row[:1, :])
                return t

            # =================================================================
            # Phase W: episodic recency/importance weights
            # =================================================================
            def rec_weight(imp_ap, ts_ap, shape, tagb):
                impt = rows.tile(shape, F32, tag=tagb + "i", name="impt")
                tst = rows.tile(shape, F32, tag=tagb + "t", name="tst")
                nc.sync.dma_start(impt[:shape[0], :], imp_ap)
                nc.sync.dma_start(tst[:shape[0], :], ts_ap)
                s = tst[:shape[0], :]
                nc.scalar.activation(s, s, AF.Copy, bias=0.0, scale=-1.0)
                nc.vector.tensor_scalar_add(s, s, 1.0)
                nc.scalar.activation(s, s, AF.Abs)
                nc.scalar.activation(s, s, AF.Exp, scale=-RECENCY)
                si = impt[:shape[0], :]
                nc.vector.tensor_scalar_add(si, si, 1.0)
                nc.vector.tensor_mul(si, si, s)
                return impt

            wfull = rec_weight(ep_imp.rearrange("(p c) -> p c", p=P),
                               ep_ts.rearrange("(p c) -> p c", p=P),
                               [P, N // P], "wf")
            wpart = rows.tile([P, 1], F32, tag="wpart", name="wpart")
            nc.vector.reduce_sum(wpart[:, :], wfull[:, :], axis=AXL.X)
            pssum = ps_sml.tile([1, 512], F32, tag="nrm", name="wsps", bufs=1)
            nc.tensor.matmul(pssum[:1, :1], ones_col[:], wpart[:, :],
                             start=True, stop=True)
            wsum = rows.tile([1, 1], F32, tag="wsum", name="wsum")
            nc.vector.tensor_copy(wsum[:1, :], pssum[:1, :1])
            nc.vector.tensor_scalar_add(wsum[:1, :], wsum[:1, :], 1e-8)
            nc.vector.reciprocal(wsum[:1, :], wsum[:1, :])
            wloc = rec_weight(ep_imp_s[None, :], ep_ts_s[None, :], [1, NL],
                              "wl")
            nc.vector.tensor_scalar(wloc[:1, :], wloc[:1, :], wsum[:1, :1],
                                    None, op0=ALU.mult)

            # =================================================================
            # Phase EK: project episodic keys, scale by w/||k||, split, AG
            # =================================================================
            with tc.tile_pool(name="ph_ek", bufs=1) as ph_ek:
                ept = ph_ek.tile([P, 2, HT, NL], BF16, tag="ept", name="ept")
                nc.sync.dma_start(ept[:], eptin)
                ekf = ph_ek.tile([P, HT, NL], F32, tag="ekf", name="ekf")
                psn_ek = ps_sml.tile([1, 512], F32, tag="nrm", name="psn_ek",
                                     bufs=1)
                for j in range(HT):
                    wc = load_wcol(ph_ek, wek_t, j)
                    ps = ps_mm.tile([P, 512], F32, tag="mm", name="ps_ek")
                    mm3(ps[:], wc, ept)
                    nc.vector.tensor_copy(ekf[:, j, :], ps[:])
                    sq = sqp.tile([P, 512], F32, tag="sq", name="sq_ek")
                    nc.scalar.square(sq[:, :], ps[:])
                    nc.tensor.matmul(psn_ek[:1, :], ones_col[:], sq[:, :],
                                     start=(j == 0), stop=(j == HT - 1))
                inv_ek = finish_inv_row(psn_ek, NL, extra_row=wloc[:1, :])
                bc_ek = sqp.tile([P, 512], F32, tag="sq", name="bc_ek")
                nc.gpsimd.partition_broadcast(bc_ek[:, :], inv_ek[:1, :])
                ek_hl = ph_ek.tile([P, 2, HT, NL], BF16, tag="ekhl",
                                   name="ek_hl")
                for j in range(HT):
                    t = sqp.tile([P, 512], F32, tag="sq", name="t_ek")
                    nc.vector.tensor_mul(t[:, :], ekf[:, j, :], bc_ek[:, :])
                    nc.scalar.activation(ek_hl[:, 0, j, :], t[:, :], AF.Copy)
                    nc.vector.tensor_sub(ek_hl[:, 1, j, :], t[:, :],
                                         ek_hl[:, 0, j, :])
                nc.sync.dma_start(
                    ag_ek_in[0:H, :].rearrange("(hi p) c -> p hi c", p=P),
                    ek_hl[:, 0, :, :])
                nc.sync.dma_start(
                    ag_ek_in[H:2 * H, :].rearrange("(hi p) c -> p hi c", p=P),
                    ek_hl[:, 1, :, :])
            nc.gpsimd.collective_compute(
                "AllGather", ALU.bypass,
                replica_groups=[list(range(NCORES))],
                ins=[ag_ek_in.opt()], outs=[ag_ek_out.opt()])

            # =================================================================
            # Phase KS: project semantic keys, split -> DRAM; norms
            # =================================================================
            bc_ks = [cst.tile([P, 512], F32, name=f"bc_ks{kc}")
                     for kc in range(4)]
            with tc.tile_pool(name="ph_ks", bufs=1) as ph_ks:
                for mc in range(ML // 512):
                    msl = slice(mc * 512, (mc + 1) * 512)
                    skt = ph_ks.tile([P, 2, HT, 512], BF16, tag="skt",
                                     name="skt", bufs=2)
                    nc.sync.dma_start(skt[:], sktin[:, :, :, msl])
                    psn = ps_sml.tile([1, 512], F32, tag="nrm",
                                      name="psn_ks", bufs=1)
                    for j in range(HT):
                        wc = load_wcol(ph_ks, wsk_t, j)
                        ps = ps_mm.tile([P, 512], F32, tag="mm",
                                        name="ps_ks")
                        mm3(ps[:], wc, skt)
                        st = sqp.tile([P, 2, 512], BF16, tag="ksst",
                                      name="ksst")
                        nc.scalar.activation(st[:, 0, :], ps[:], AF.Copy)
                        nc.vector.tensor_sub(st[:, 1, :], ps[:], st[:, 0, :])
                        nc.sync.dma_start(
                            ks_dram[j * P:(j + 1) * P, msl], st[:, 0, :])
                        nc.sync.dma_start(
                            ks_dram[H + j * P:H + (j + 1) * P, msl],
                            st[:, 1, :])
                        sq = sqp.tile([P, 512], F32, tag="sq", name="sq_ks")
                        nc.scalar.square(sq[:, :], ps[:])
                        nc.tensor.matmul(psn[:1, :], ones_col[:], sq[:, :],
                                         start=(j == 0), stop=(j == HT - 1))
                    inv = finish_inv_row(psn, 512)
                    nc.gpsimd.partition_broadcast(bc_ks[mc][:, :],
                                                  inv[:1, :512])

            with tc.tile_pool(name="ph_acc", bufs=1) as ph_acc:
                with tc.tile_pool(name="ph_qhl", bufs=1) as ph_qhl:
                    # =========================================================
                    # Phase Q: project queries, split (unscaled), norms
                    # =========================================================
                    q_hl = ph_qhl.tile([P, 2, HT, BL], BF16, tag="qhl",
                                       name="q_hl")
                    with tc.tile_pool(name="ph_qt", bufs=1) as ph_qt:
                        qt = ph_qt.tile([P, 2, HT, BL], BF16, tag="qt",
                                        name="qt")
                        nc.sync.dma_start(qt[:], qtin)
                        psn_q = ps_sml.tile([1, 512], F32, tag="nrm",
                                            name="psn_q", bufs=1)
                        for j in range(HT):
                            wc = load_wcol(ph_qhl, wq_t, j)
                            ps = ps_mm.tile([P, 512], F32, tag="mm",
                                            name="ps_q")
                            mm3(ps[:], wc, qt)
                            nc.scalar.activation(q_hl[:, 0, j, :], ps[:],
                                                 AF.Copy)
                            nc.vector.tensor_sub(q_hl[:, 1, j, :], ps[:],
                                                 q_hl[:, 0, j, :])
                            sq = sqp.tile([P, 512], F32, tag="sq", name="sq_q")
                            nc.scalar.square(sq[:, :], ps[:])
                            nc.tensor.matmul(psn_q[:1, :], ones_col[:],
                                             sq[:, :], start=(j == 0),
                                             stop=(j == HT - 1))
                        inv_q = finish_inv_row(psn_q, BL)
                        nc.sync.dma_start(bounce[0:1, :], inv_q[:1, :])

                    # =========================================================
                    # Phase QS: semantic query projection (unscaled)
                    # =========================================================
                    with tc.tile_pool(name="ph_qs", bufs=1) as ph_qs:
                        qs_hl = ph_qs.tile([P, 2, HT, BL], BF16, tag="qshl",
                                           name="qs_hl")
                        psn_qs = ps_sml.tile([1, 512], F32, tag="nrm",
                                             name="psn_qs", bufs=1)
                        for j in range(HT):
                            wc = load_wcol(ph_qhl, wsq_t, j)
                            ps = ps_mm.tile([P, 512], F32, tag="mm",
                                            name="ps_qs")
                            mm3(ps[:], wc, q_hl)
                            nc.scalar.activation(qs_hl[:, 0, j, :], ps[:],
                                                 AF.Copy)
                            nc.vector.tensor_sub(qs_hl[:, 1, j, :], ps[:],
                                                 qs_hl[:, 0, j, :])
                            sq = sqp.tile([P, 512], F32, tag="sq",
                                          name="sq_qs")
                            nc.scalar.square(sq[:, :], ps[:])
                            nc.tensor.matmul(psn_qs[:1, :], ones_col[:],
                                             sq[:, :], start=(j == 0),
                                             stop=(j == HT - 1))
                        inv_qs = finish_inv_row(psn_qs, BL)
                        nc.sync.dma_start(bounce[1:2, :], inv_qs[:1, :])
                        nc.sync.dma_start(
                            ag_qs_in[0:H, :].rearrange("(hi p) c -> p hi c",
                                                       p=P),
                            qs_hl[:, 0, :, :])
                        nc.sync.dma_start(
                            ag_qs_in[H:2 * H, :].rearrange(
                                "(hi p) c -> p hi c", p=P),
                            qs_hl[:, 1, :, :])
                    nc.gpsimd.collective_compute(
                        "AllGather", ALU.bypass,
                        replica_groups=[list(range(NCORES))],
                        ins=[ag_qs_in.opt()], outs=[ag_qs_out.opt()])

                    invq_p = cst.tile([P, NBT], F32, name="invq_p")
                    invqs_p = cst.tile([P, NBT], F32, name="invqs_p")
                    nc.sync.dma_start(
                        invq_p[:, :],
                        bounce[0:1, :].rearrange("o (t p) -> (o p) t", p=P))
                    nc.sync.dma_start(
                        invqs_p[:, :],
                        bounce[1:2, :].rearrange("o (t p) -> (o p) t", p=P))

                    # --- work attention + gate precompute ---
                    wsT = cst.tile([P, HT, S], BF16, name="wsT")
                    nc.sync.dma_start(wsT[:], wsT_b)
                    gw1 = cst.tile([P, HT, 64], BF16, name="gw1")
                    nc.sync.dma_start(gw1[:], gw1_b)
                    gw2 = cst.tile([64, 3], BF16, name="gw2")
                    nc.sync.dma_start(gw2[:, :], gw2_b)
                    b1bc = bcast_row_dram(gate_b1[None, :], 64, "b1bc")
                    b2bc = bcast_row_dram(gate_b2[None, :], 3, "b2bc")
                    kb_bc = bcast_row_dram(key_base[None, :], 1, "kb_bc")

                    inv_sqrt_h = 1.0 / math.sqrt(H)
                    ewT_pre = []
                    gw_pre = []
                    for bt in range(NBT):
                        qsl = slice(bt * P, (bt + 1) * P)
                        psw = ps_sml.tile([P, S], F32, tag="sml", name="pswk", bufs=1)
                        for hi in range(HT):
                            nc.tensor.matmul(
                                psw[:, :S], q_hl[:, 0, hi, qsl], wsT[:, hi, :],
                                start=(hi == 0), stop=(hi == HT - 1))
                        wmax = tiny.tile([P, 1], F32, tag="c1", name="wmax")
                        nc.vector.reduce_max(wmax[:, :], psw[:, :S],
                                             axis=AXL.X)
                        nc.vector.tensor_scalar_mul(wmax[:, :], wmax[:, :],
                                                    -inv_sqrt_h)
                        ew = tiny.tile([P, S], F32, tag="c64", name="ew")
                        nc.scalar.activation(ew[:, :], psw[:, :S], AF.Exp,
                                             bias=wmax[:, :1],
                                             scale=inv_sqrt_h)
                        zw = tiny.tile([P, 1], F32, tag="c1", name="zw")
                        nc.vector.reduce_sum(zw[:, :], ew[:, :], axis=AXL.X)
                        nc.vector.reciprocal(zw[:, :], zw[:, :])
                        nc.vector.tensor_scalar(ew[:, :], ew[:, :],
                                                zw[:, :1], None, op0=ALU.mult)
                        pset = ps_tr.tile([S, P], F32, tag="tr", name="ewtp")
                        nc.tensor.transpose(out=pset[:S, :], in_=ew[:, :],
                                            identity=ident[:])
                        ewT = cst.tile([S, P], BF16, name=f"ewT{bt}")
                        nc.vector.tensor_copy(ewT[:, :], pset[:S, :])
                        ewT_pre.append(ewT)

                        psg = ps_sml.tile([P, 64], F32, tag="sml", name="psg", bufs=1)
                        for hi in range(HT):
                            nc.tensor.matmul(
                                psg[:, :64], q_hl[:, 0, hi, qsl],
                                gw1[:, hi, :],
                                start=(hi == 0), stop=(hi == HT - 1))
                        hid = tiny.tile([P, 64], F32, tag="c64", name="hid")
                        nc.vector.tensor_add(hid[:, :], psg[:, :64],
                                             b1bc[:, :])
                        nc.scalar.activation(hid[:, :], hid[:, :], AF.Silu)
                        psht = ps_tr.tile([64, P], F32, tag="tr", name="hidtp")
                        nc.tensor.transpose(out=psht[:64, :], in_=hid[:, :],
                                            identity=ident[:])
                        hidT = tiny.tile([64, P], BF16, tag="c128",
                                         name="hidT")
                        nc.vector.tensor_copy(hidT[:, :], psht[:64, :])
                        psg2 = ps_sml.tile([P, 3], F32, tag="sml", name="psg2", bufs=1)
                        nc.tensor.matmul(psg2[:, :3], hidT[:, :], gw2[:, :],
                                         start=True, stop=True)
                        gl = cst.tile([P, 3], F32, name=f"gl{bt}")
                        nc.vector.tensor_add(gl[:, :], psg2[:, :3], b2bc[:, :])
                        gmax = tiny.tile([P, 1], F32, tag="c1", name="gmax")
                        nc.vector.reduce_max(gmax[:, :], gl[:, :], axis=AXL.X)
                        nc.vector.tensor_scalar_mul(gmax[:, :], gmax[:, :],
                                                    -1.0)
                        nc.scalar.activation(gl[:, :], gl[:, :], AF.Exp,
                                             bias=gmax[:, :1])
                        gz = tiny.tile([P, 1], F32, tag="c1", name="gz")
                        nc.vector.reduce_sum(gz[:, :], gl[:, :], axis=AXL.X)
                        nc.vector.reciprocal(gz[:, :], gz[:, :])
                        nc.vector.tensor_scalar(gl[:, :], gl[:, :],
                                                gz[:, :1], None, op0=ALU.mult)
                        gw_pre.append(gl)

                    # =========================================================
                    # Phase SIM-E: own queries x all episodic keys
                    # =========================================================
                    cand_v_e = [cst.tile([P, 128], F32, name=f"cve{bt}")
                                for bt in range(NBT)]
                    cand_i_e = [cst.tile([P, 128], F32, name=f"cie{bt}")
                                for bt in range(NBT)]
                    with tc.tile_pool(name="ph_se", bufs=2) as ph_se:
                        for slab in range(NCORES):
                            base = slab * 2 * H
                            for khalf in range(2):
                                csl = slice(khalf * 256, (khalf + 1) * 256)
                                ekg = ph_se.tile([P, 2, HT, 256], BF16,
                                                 tag="ekg", name="ekg")
                                nc.sync.dma_start(
                                    ekg[:],
                                    ag_ek_out[base:base + 2 * H,
                                              csl].rearrange(
                                        "(s hi p) c -> p s hi c", p=P, s=2))
                                cid = 2 * slab + khalf
                                for bt in range(NBT):
                                    qsl = slice(bt * P, (bt + 1) * P)
                                    ps = ps_mm.tile([P, 512], F32, tag="mm",
                                                    name="ps_se")
                                    mm3(ps[:, :256], q_hl, ekg, s_sl=qsl)
                                    sc = simcp.tile([P, 256], F32, tag="sime",
                                                    name="sc_e")
                                    nc.vector.tensor_copy(sc[:], ps[:, :256])
                                    mx = tiny.tile([P, 8], F32, tag="mx",
                                                   name="mx_e")
                                    mi = tiny.tile([P, 8], U32, tag="mi",
                                                   name="mi_e")
                                    nc.vector.max(out=mx[:], in_=sc[:])
                                    nc.vector.max_index(out=mi[:],
                                                        in_max=mx[:],
                                                        in_values=sc[:])
                                    nc.vector.tensor_copy(
                                        cand_v_e[bt][:,
                                                     cid * 8:(cid + 1) * 8],
                                        mx[:])
                                    mif = tiny.tile([P, 8], F32, tag="mif",
                                                    name="mif_e")
                                    nc.vector.tensor_copy(mif[:], mi[:])
                                    nc.vector.tensor_scalar_add(
                                        cand_i_e[bt][:,
                                                     cid * 8:(cid + 1) * 8],
                                        mif[:], float(cid * 256))

                    # --- episodic top-8 merge + gather + weighted sum ---
                    acc_e_b = [ph_acc.tile([P, H], BF16, tag=f"acce{bt}",
                                           name=f"acce{bt}")
                               for bt in range(NBT)]
                    for bt in range(NBT):
                        top8 = tiny.tile([P, 8], F32, tag="c8", name="top8")
                        nc.vector.max(out=top8[:], in_=cand_v_e[bt][:])
                        idxf = tiny.tile([P, 8], F32, tag="c8b", name="idxf")
                        eqm = simcp.tile([P, 128], F32, tag="eqm", name="eqm")
                        for kk in range(EP_K):
                            nc.vector.tensor_scalar(
                                eqm[:, :], cand_v_e[bt][:],
                                top8[:, kk:kk + 1], None, op0=ALU.is_equal)
                            nc.vector.tensor_tensor(
                                out=eqm[:, :], in0=eqm[:, :],
                                in1=cand_i_e[bt][:], op=ALU.mult)
                            nc.vector.reduce_sum(idxf[:, kk:kk + 1],
                                                 eqm[:, :], axis=AXL.X)
                        idxu = tiny.tile([P, 8], U32, tag="c8u", name="idxu")
                        nc.vector.tensor_copy(idxu[:, :], idxf[:, :])
                        sc8 = tiny.tile([P, 8], F32, tag="c8c", name="sc8")
                        nc.vector.tensor_scalar(
                            sc8[:, :], top8[:, :], invq_p[:, bt:bt + 1], None,
                            op0=ALU.mult)
                        negm = tiny.tile([P, 1], F32, tag="c1", name="negm")
                        nc.vector.tensor_scalar_mul(negm[:, :], sc8[:, 0:1],
                                                    -1.0)
                        nc.scalar.activation(sc8[:, :], sc8[:, :], AF.Exp,
                                             bias=negm[:, :1])
                        zs = tiny.tile([P, 1], F32, tag="c1", name="zs")
                        nc.vector.reduce_sum(zs[:, :], sc8[:, :], axis=AXL.X)
                        nc.vector.reciprocal(zs[:, :], zs[:, :])
                        nc.vector.tensor_scalar(zs[:, :], zs[:, :],
                                                gw_pre[bt][:, 1:2], None,
                                                op0=ALU.mult)
                        nc.vector.tensor_scalar(sc8[:, :], sc8[:, :],
                                                zs[:, :1], None, op0=ALU.mult)
                        acc = simcp.tile([P, H], F32, tag="acc", name="acc_e",
                                         bufs=2)
                        nc.vector.memset(acc[:, :], 0.0)
                        for kk in range(EP_K):
                            g = gath.tile([P, H], BF16, tag="g", name="g_e")
                            nc.gpsimd.indirect_dma_start(
                                out=g[:, :], out_offset=None, in_=ep_store_b,
                                in_offset=bass.IndirectOffsetOnAxis(
                                    ap=idxu[:, kk:kk + 1], axis=0))
                            nc.vector.scalar_tensor_tensor(
                                out=acc[:, :], in0=g[:, :],
                                scalar=sc8[:, kk:kk + 1],
                                in1=acc[:, :], op0=ALU.mult, op1=ALU.add)
                        nc.vector.tensor_copy(acc_e_b[bt][:, :], acc[:, :])

                # ==== ph_qhl closed: q_hl freed ====
                # =============================================================
                # Phase SIM-S: ALL queries x local semantic keys (kc-outer)
                # =============================================================
                with tc.tile_pool(name="ph_ss", bufs=1) as ph_ss:
                    cand_sv = ph_ss.tile([P, 32 * 32], F32, tag="csv",
                                         name="cand_sv")
                    cand_si = ph_ss.tile([P, 32 * 32], F32, tag="csi",
                                         name="cand_si")
                    for kc in range(4):
                        msl = slice(kc * 512, (kc + 1) * 512)
                        ksc = ph_ss.tile([P, 2, HT, 512], BF16, tag="ksc",
                                         name="ksc")
                        nc.sync.dma_start(
                            ksc[:],
                            ks_dram[:, msl].rearrange(
                                "(s hi p) c -> p s hi c", p=P, s=2))
                        for rq2 in range(16):
                            slabq = rq2 // 2
                            base = slabq * 2 * H
                            col0 = (rq2 % 2) * 256
                            qsg = ph_ss.tile([P, 2, HT, 256], BF16, tag="qsg",
                                             name="qsg", bufs=2)
                            nc.sync.dma_start(
                                qsg[:],
                                ag_qs_out[base:base + 2 * H,
                                          col0:col0 + 256].rearrange(
                                    "(s hi p) c -> p s hi c", p=P, s=2))
                            for rq in range(2):
                                rqt = rq2 * 2 + rq
                                qssl = slice(rq * P, (rq + 1) * P)
                                ps = ps_mm.tile([P, 512], F32, tag="mm",
                                                name="ps_ss")
                                mm3(ps[:], qsg, ksc, s_sl=qssl)
                                sc = simcp.tile([P, 512], F32, tag="scs",
                                                name="sc_s")
                                nc.vector.tensor_mul(sc[:, :], ps[:],
                                                     bc_ks[kc][:, :])
                                mx = tiny.tile([P, 8], F32, tag="mx",
                                               name="mx_s")
                                mi = tiny.tile([P, 8], U32, tag="mi",
                                               name="mi_s")
                                nc.vector.max(out=mx[:], in_=sc[:])
                                nc.vector.max_index(out=mi[:], in_max=mx[:],
                                                    in_values=sc[:])
                                wsl = slice(rqt * 32 + kc * 8,
                                            rqt * 32 + (kc + 1) * 8)
                                nc.vector.tensor_copy(cand_sv[:, wsl], mx[:])
                                mif = tiny.tile([P, 8], F32, tag="mif",
                                                name="mif_s")
                                nc.vector.tensor_copy(mif[:], mi[:])
                                nc.vector.tensor_scalar_add(
                                    cand_si[:, wsl], mif[:], float(kc * 512))

                    # local top-4 per query, global index, ship via a2a
                    for rqt in range(32):
                        wsl = slice(rqt * 32, (rqt + 1) * 32)
                        top8 = tiny.tile([P, 8], F32, tag="c8", name="top8l")
                        nc.vector.max(out=top8[:], in_=cand_sv[:, wsl])
                        idxf = tiny.tile([P, 8], F32, tag="c8b", name="idxfl")
                        eqm = simcp.tile([P, 32], F32, tag="eqs", name="eqml")
                        for kk in range(SEM_K):
                            nc.vector.tensor_scalar(
                                eqm[:, :], cand_sv[:, wsl],
                                top8[:, kk:kk + 1], None, op0=ALU.is_equal)
                            nc.vector.tensor_tensor(
                                out=eqm[:, :], in0=eqm[:, :],
                                in1=cand_si[:, wsl], op=ALU.mult)
                            nc.vector.reduce_sum(idxf[:, kk:kk + 1],
                                                 eqm[:, :], axis=AXL.X)
                        p4 = tiny.tile([P, 8], F32, tag="p4", name="p4")
                        nc.vector.tensor_copy(p4[:, 0:4], top8[:, 0:4])
                        nc.vector.tensor_scalar(
                            p4[:, 4:8], idxf[:, 0:4], kb_bc[:, 0:1], None,
                            op0=ALU.add)
                        nc.sync.dma_start(cand_in[rqt * P:(rqt + 1) * P, :],
                                          p4[:, :])
                nc.gpsimd.collective_compute(
                    "AllToAll", ALU.bypass,
                    replica_groups=[list(range(NCORES))],
                    ins=[cand_in.opt()], outs=[cand_out.opt()])

                # =============================================================
                # Phase FINAL
                # =============================================================
                with tc.tile_pool(name="fin", bufs=1) as fin:
                    def transpose_b(src_b, dst):
                        for hi in range(HT):
                            pst = ps_tr.tile([P, P], BF16, tag="trb16",
                                             name="trp", bufs=2)
                            nc.tensor.transpose(
                                out=pst[:], in_=src_b[:, hi * P:(hi + 1) * P],
                                identity=ident_b[:])
                            nc.vector.tensor_copy(dst[:, hi, :], pst[:])

                    def val_stage(w_ap, accT_list, out_tiles, mode,
                                  gscale=None):
                        for jc in range(4):
                            wv = fin.tile([P, HT, 512], BF16, tag="wv",
                                          name="wv", bufs=2)
                            nc.sync.dma_start(wv[:], w_ap[jc])
                            jsl = slice(jc * 512, (jc + 1) * 512)
                            for bt in range(NBT):
                                ps = ps_mm.tile([P, 512], F32, tag="mm",
                                                name="ps_v")
                                for hi in range(HT):
                                    nc.tensor.matmul(
                                        ps[:], accT_list[bt][:, hi, :],
                                        wv[:, hi, :], start=(hi == 0),
                                        stop=(hi == HT - 1))
                                if mode == "set":
                                    nc.vector.tensor_copy(
                                        out_tiles[bt][:, jsl], ps[:])
                                else:
                                    nc.vector.tensor_add(
                                        out_tiles[bt][:, jsl],
                                        out_tiles[bt][:, jsl], ps[:])

                    # e chain: tmp_e = acc_e @ W_ev
                    accT = [fin.tile([P, HT, P], BF16, tag="accT",
                                     name=f"accT{bt}", bufs=4)
                            for bt in range(NBT)]
                    for bt in range(NBT):
                        transpose_b(acc_e_b[bt], accT[bt])
                    tmp_e = [fin.tile([P, H], BF16, tag="t16",
                                      name=f"tmpe{bt}", bufs=4)
                             for bt in range(NBT)]
                    val_stage(wev_t, accT, tmp_e, "set")
                    accT2 = [fin.tile([P, HT, P], BF16, tag="accT",
                                      name=f"accT2{bt}", bufs=4)
                             for bt in range(NBT)]
                    for bt in range(NBT):
                        transpose_b(tmp_e[bt], accT2[bt])

                    # bl = gl0 * w_out
                    bl = [fin.tile([P, H], F32, tag="f32b", name=f"bl{bt}",
                                   bufs=4)
                          for bt in range(NBT)]
                    for jc in range(4):
                        wvw = fin.tile([S, 512], BF16, tag="wvw", name="wvw",
                                       bufs=2)
                        nc.sync.dma_start(wvw[:S, :],
                                          work_b[:, jc * 512:(jc + 1) * 512])
                        jsl = slice(jc * 512, (jc + 1) * 512)
                        for bt in range(NBT):
                            ps = ps_mm.tile([P, 512], F32, tag="mm",
                                            name="ps_w")
                            nc.tensor.matmul(ps[:], ewT_pre[bt][:, :],
                                             wvw[:S, :], start=True,
                                             stop=True)
                            nc.vector.tensor_scalar(
                                bl[bt][:, jsl], ps[:], gw_pre[bt][:, 0:1],
                                None, op0=ALU.mult)

                    # bl += tmp_e @ W_eo
                    val_stage(weo_t, accT2, bl, "add")

                    # --- semantic merge + gather (after AllToAll) ---
                    acc_s_b = [ph_acc.tile([P, H], BF16, tag=f"accs{bt}",
                                           name=f"accs{bt}")
                               for bt in range(NBT)]
                    for bt in range(NBT):
                        c32v = simcp.tile([P, 32], F32, tag="eqs",
                                          name="c32v")
                        c32i = simcp.tile([P, 32], F32, tag="eqs2",
                                          name="c32i")
                        for r in range(NCORES):
                            c8 = tiny.tile([P, 8], F32, tag="p4", name="c8in")
                            nc.sync.dma_start(
                                c8[:, :],
                                cand_out[r * BL + bt * P:
                                         r * BL + (bt + 1) * P, :])
                            nc.vector.tensor_copy(c32v[:, r * 4:(r + 1) * 4],
                                                  c8[:, 0:4])
                            nc.vector.tensor_copy(c32i[:, r * 4:(r + 1) * 4],
                                                  c8[:, 4:8])
                        top8 = tiny.tile([P, 8], F32, tag="c8", name="top8s")
                        nc.vector.max(out=top8[:], in_=c32v[:])
                        idxf = tiny.tile([P, 8], F32, tag="c8b", name="idxfs")
                        eqs = simcp.tile([P, 32], F32, tag="eqs3", name="eqs")
                        for kk in range(SEM_K):
                            nc.vector.tensor_scalar(
                                eqs[:, :], c32v[:, :], top8[:, kk:kk + 1],
                                None, op0=ALU.is_equal)
                            nc.vector.tensor_tensor(out=eqs[:, :],
                                                    in0=eqs[:, :],
                                                    in1=c32i[:, :],
                                                    op=ALU.mult)
                            nc.vector.reduce_sum(idxf[:, kk:kk + 1],
                                                 eqs[:, :], axis=AXL.X)
                        idxu = tiny.tile([P, 8], U32, tag="c8u", name="idxus")
                        nc.vector.tensor_copy(idxu[:, 0:4], idxf[:, 0:4])
                        sc4 = tiny.tile([P, 4], F32, tag="c4", name="sc4")
                        nc.vector.tensor_scalar(
                            sc4[:, :], top8[:, 0:4], invqs_p[:, bt:bt + 1],
                            None, op0=ALU.mult)
                        negm = tiny.tile([P, 1], F32, tag="c1", name="negms")
                        nc.vector.tensor_scalar_mul(negm[:, :], sc4[:, 0:1],
                                                    -1.0)
                        nc.scalar.activation(sc4[:, :], sc4[:, :], AF.Exp,
                                             bias=negm[:, :1])
                        zs = tiny.tile([P, 1], F32, tag="c1", name="zss")
                        nc.vector.reduce_sum(zs[:, :], sc4[:, :], axis=AXL.X)
                        nc.vector.reciprocal(zs[:, :], zs[:, :])
                        nc.vector.tensor_scalar(zs[:, :], zs[:, :],
                                                gw_pre[bt][:, 2:3], None,
                                                op0=ALU.mult)
                        nc.vector.tensor_scalar(sc4[:, :], sc4[:, :],
                                                zs[:, :1], None, op0=ALU.mult)
                        acc = simcp.tile([P, H], F32, tag="acc", name="acc_s",
                                         bufs=2)
                        nc.vector.memset(acc[:, :], 0.0)
                        for kk in range(SEM_K):
                            g = gath.tile([P, H], BF16, tag="g", name="g_s")
                            nc.gpsimd.indirect_dma_start(
                                out=g[:, :], out_offset=None,
                                in_=sem_values_b,
                                in_offset=bass.IndirectOffsetOnAxis(
                                    ap=idxu[:, kk:kk + 1], axis=0))
                            nc.vector.scalar_tensor_tensor(
                                out=acc[:, :], in0=g[:, :],
                                scalar=sc4[:, kk:kk + 1],
                                in1=acc[:, :], op0=ALU.mult, op1=ALU.add)
                        nc.vector.tensor_copy(acc_s_b[bt][:, :], acc[:, :])

                    # bl += acc_s @ W_so
                    accT_s = [fin.tile([P, HT, P], BF16, tag="accT",
                                       name=f"accTs{bt}", bufs=4)
                              for bt in range(NBT)]
                    for bt in range(NBT):
                        transpose_b(acc_s_b[bt], accT_s[bt])
                    val_stage(wso_t, accT_s, bl, "add")

                    # xo = bl @ W_ro; out = LN(xo)*gamma+beta
                    blb = [fin.tile([P, H], BF16, tag="t16", name=f"blb{bt}",
                                    bufs=4)
                           for bt in range(NBT)]
                    for bt in range(NBT):
                        nc.vector.tensor_copy(blb[bt][:, :], bl[bt][:, :])
                    accT_bl = [fin.tile([P, HT, P], BF16, tag="accT",
                                        name=f"accTb{bt}", bufs=4)
                               for bt in range(NBT)]
                    for bt in range(NBT):
                        transpose_b(blb[bt], accT_bl[bt])
                    xo = [fin.tile([P, H], F32, tag="f32b", name=f"xo{bt}",
                                   bufs=4)
                          for bt in range(NBT)]
                    val_stage(wro_t, accT_bl, xo, "set")

                    for bt in range(NBT):
                        x = xo[bt]
                        mu = tiny.tile([P, 1], F32, tag="c1", name="mu")
                        nc.vector.reduce_sum(mu[:, :], x[:, :], axis=AXL.X)
                        nc.vector.tensor_scalar_mul(mu[:, :], mu[:, :],
                                                    -1.0 / H)
                        nc.vector.tensor_scalar(x[:, :], x[:, :], mu[:, :1],
                                                None, op0=ALU.add)
                        sqx = simcp.tile([P, H], F32, tag="acc", name="sqx",
                                         bufs=2)
                        vs = tiny.tile([P, 1], F32, tag="c1", name="vs")
                        nc.scalar.activation(sqx[:, :], x[:, :], AF.Square,
                                             accum_out=vs[:, :1])
                        nc.vector.tensor_scalar_mul(vs[:, :], vs[:, :],
                                                    1.0 / H)
                        nc.vector.tensor_scalar_add(vs[:, :], vs[:, :],
                                                    LN_EPS)
                        nc.scalar.sqrt(vs[:, :], vs[:, :])
                        nc.vector.reciprocal(vs[:, :], vs[:, :])
                        nc.vector.tensor_scalar(x[:, :], x[:, :], vs[:, :1],
                                                None, op0=ALU.mult)
                        for jc in range(4):
                            jsl = slice(jc * 512, (jc + 1) * 512)
                            gbch = sqp.tile([P, 512], F32, tag="sq",
                                            name="gbch")
                            grow = rows.tile([1, 512], F32, tag="crow",
                                             name="grow")
                            nc.sync.dma_start(grow[:1, :],
                                              ln_gamma[None, jsl])
                            nc.gpsimd.partition_broadcast(gbch[:, :],
                                                          grow[:1, :])
                            nc.vector.tensor_mul(x[:, jsl], x[:, jsl],
                                                 gbch[:, :])
                            bbch = sqp.tile([P, 512], F32, tag="sq",
                                            name="bbch")
                            brow = rows.tile([1, 512], F32, tag="crow",
                                             name="brow")
                            nc.sync.dma_start(brow[:1, :],
                                              ln_beta[None, jsl])
                            nc.gpsimd.partition_broadcast(bbch[:, :],
                                                          brow[:1, :])
                            nc.vector.tensor_add(x[:, jsl], x[:, jsl],
                                                 bbch[:, :])
                        nc.sync.dma_start(out_s[bt * P:(bt + 1) * P, :],
                                          x[:, :])

    nc.finalize()
    return nc


_NC_CACHE = None


def _bf16_split(x):
    h = x.astype(ml_dtypes.bfloat16)
    l = (x - h.astype(np.float32)).astype(ml_dtypes.bfloat16)
    return h, l


def _tile_sel_weight(w):
    """[H, H] f32 -> [j, p, 2, hi, 128] bf16 hi/lo tiled."""
    h, l = _bf16_split(w)
    out = np.empty((HT, P, 2, HT, P), dtype=ml_dtypes.bfloat16)
    hr = h.reshape(HT, P, HT, P)   # [hi, p, j, c]
    lr = l.reshape(HT, P, HT, P)
    out[:, :, 0] = hr.transpose(2, 1, 0, 3)
    out[:, :, 1] = lr.transpose(2, 1, 0, 3)
    return np.ascontiguousarray(out)


def _tile_val_weight(w):
    """[H, H] f32 -> [jc, p, hi, 512] bf16."""
    b = w.astype(ml_dtypes.bfloat16)
    r = b.reshape(HT, P, 4, 512)   # [hi, p, jc, c]
    return np.ascontiguousarray(r.transpose(2, 1, 0, 3))


def _split_T(x):
    """[R, H] f32 -> [p, 2, hi, R] bf16 (transposed hi/lo)."""
    h, l = _bf16_split(x)
    R = x.shape[0]
    out = np.empty((P, 2, HT, R), dtype=ml_dtypes.bfloat16)
    out[:, 0] = h.T.reshape(HT, P, R).transpose(1, 0, 2)
    out[:, 1] = l.T.reshape(HT, P, R).transpose(1, 0, 2)
    return np.ascontiguousarray(out)


def kernel(**inputs) -> np.ndarray:
    global _NC_CACHE
    if _NC_CACHE is None:
        _NC_CACHE = build()
    nc = _NC_CACHE

    f32 = lambda x: np.ascontiguousarray(np.asarray(x), dtype=np.float32)
    query = f32(inputs["query"])
    ep_store = f32(inputs["ep_store"])
    sem_keys = f32(inputs["sem_keys"])
    work_slots = f32(inputs["work_slots"])

    shared = {
        "wq_t": _tile_sel_weight(f32(inputs["W_query"])),
        "wek_t": _tile_sel_weight(f32(inputs["W_ek"])),
        "wsq_t": _tile_sel_weight(f32(inputs["W_sq"])),
        "wsk_t": _tile_sel_weight(f32(inputs["W_sk"])),
        "wev_t": _tile_val_weight(f32(inputs["W_ev"])),
        "weo_t": _tile_val_weight(f32(inputs["W_eo"])),
        "wso_t": _tile_val_weight(f32(inputs["W_so"])),
        "wro_t": _tile_val_weight(f32(inputs["W_ro"])),
        "ep_store_b": ep_store.astype(ml_dtypes.bfloat16),
        "sem_values_b": f32(inputs["sem_values"]).astype(ml_dtypes.bfloat16),
        "wsT_b": np.ascontiguousarray(
            work_slots.T.astype(ml_dtypes.bfloat16).reshape(HT, P, S)
            .transpose(1, 0, 2)),
        "work_b": work_slots.astype(ml_dtypes.bfloat16),
        "gw1_b": np.ascontiguousarray(
            f32(inputs["gate_W1"]).astype(ml_dtypes.bfloat16)
            .reshape(HT, P, 64).transpose(1, 0, 2)),
        "gw2_b": f32(inputs["gate_W2"]).astype(ml_dtypes.bfloat16),
        "ep_imp": f32(inputs["ep_importance"]),
        "ep_ts": f32(inputs["ep_timestamps"]),
        "gate_b1": f32(inputs["gate_b1"]),
        "gate_b2": f32(inputs["gate_b2"]),
        "ln_gamma": f32(inputs["ln_gamma"]),
        "ln_beta": f32(inputs["ln_beta"]),
    }

    in_maps = []
    for c in range(NCORES):
        m = dict(shared)
        m["qtin"] = _split_T(query[c * BL:(c + 1) * BL])
        m["eptin"] = _split_T(ep_store[c * NL:(c + 1) * NL])
        m["sktin"] = _split_T(sem_keys[c * ML:(c + 1) * ML])
        m["ep_imp_s"] = f32(inputs["ep_importance"][c * NL:(c + 1) * NL])
        m["ep_ts_s"] = f32(inputs["ep_timestamps"][c * NL:(c + 1) * NL])
        m["key_base"] = np.array([c * ML], dtype=np.float32)
        in_maps.append(m)

    res = run_bass_kernel_spmd(nc, in_maps, core_ids=list(range(NCORES)))
    return np.concatenate([res.results[c]["out_s"] for c in range(NCORES)],
                          axis=0)
